# revision 16
# baseline (speedup 1.0000x reference)
"""Trainium2 Bass kernel for BipartiteGCN (8 NeuronCores, SPMD) — v2.

Strategy:
 - Node rows sharded 8 ways; edges sharded by DESTINATION range, grouped by
   (dst_block, src_bucket) segments with src-sorted order inside.
 - Edge loop (edge-major [e, f] tiles of 128 edges):
   dma_gather lp[src] rows (bf16), one-hot matmul broadcasts rp[dst] into
   PSUM, one DVE add per 4 tiles, grouped bn_stats, ScalarE fused
   (x-mu)*rstd + LeakyReLU, acc^T matmul (feature-major segment sum,
   4 dst-blocks per PSUM bank).
 - Scatter-mean denominators precomputed on host (no ones column).
 - Post-conv + embed MLPs run feature-major: batched N=512 matmuls with
   stationary weights, LN via colsum matmuls + row math, rank-1 bias /
   mean-correction matmuls, biases as per-partition columns. No PE
   transposes anywhere.
 - lp tables all-gathered (bf16); heads interleaved with conv2 post.
"""

import os
import sys

for _p in ("/opt/trn_rl_repo",):
    if _p not in sys.path:
        sys.path.insert(0, _p)

import numpy as np
import ml_dtypes

import concourse.bass as bass
import concourse.bacc as bacc
import concourse.mybir as mybir
from concourse import tile, library_config
from concourse.bass_utils import run_bass_kernel_spmd

BF16 = ml_dtypes.bfloat16
F32 = np.float32
NCORES = 8
EMB = 128
CHUNK_TILES = 16
HI_BASE = 32768
EPS = 1e-5
SLOPE = 0.01

dt = mybir.dt
AL = mybir.AluOpType
LR = mybir.ActivationFunctionType.Lrelu
SQ = mybir.ActivationFunctionType.Sqrt
ARS = mybir.ActivationFunctionType.Abs_reciprocal_sqrt


def _wrap_idx(idx_i16):
    n = idx_i16.shape[0]
    assert n % 16 == 0
    w = idx_i16.reshape(n // 16, 16).T
    return np.tile(w, (8, 1)).copy()


def _col(v):
    return np.asarray(v, F32)[:, None].copy()


def _bcast_row(v, rows=128):
    return np.broadcast_to(np.asarray(v, F32)[None, :], (rows, v.shape[0])).copy()


class ConvPrep:
    """Per-conv edge data: segments of (dst_block, src_bucket), src-sorted."""

    def __init__(self, dst, src, n_dst, n_src, dst_per_core):
        self.n_dst_local = dst_per_core
        self.nblocks = -(-dst_per_core // 128)
        self.two_buckets = n_src > HI_BASE
        nb = self.nblocks
        nu = 2 if self.two_buckets else 1

        core = dst // dst_per_core
        dloc_all = dst - core * dst_per_core
        block_all = dloc_all // 128

        self.counts = np.zeros((NCORES, dst_per_core), np.int64)
        for c in range(NCORES):
            m = core == c
            self.counts[c] = np.bincount(dloc_all[m], minlength=dst_per_core)

        per = [[[None] * nu for _ in range(nb)] for _ in range(NCORES)]
        for c in range(NCORES):
            m = core == c
            d_c = dloc_all[m]
            s_c = src[m]
            b_c = block_all[m]
            u_c = (s_c >= HI_BASE).astype(np.int8) if self.two_buckets else np.zeros(
                len(s_c), np.int8
            )
            for bq in range(nb):
                for u in range(nu):
                    mu = (b_c == bq) & (u_c == u)
                    order = np.argsort(s_c[mu], kind="stable")
                    per[c][bq][u] = (s_c[mu][order], d_c[mu][order])

        self.ntiles = np.zeros((nb, nu), np.int64)
        for bq in range(nb):
            for u in range(nu):
                mx = max(len(per[c][bq][u][0]) for c in range(NCORES))
                self.ntiles[bq, u] = -(-mx // 128) if mx > 0 else 0

        etot = int(self.ntiles.sum()) * 128
        self.etot = etot

        self.src_idx = np.zeros((NCORES, etot), np.int16)
        self.dstrel = np.full((NCORES, etot), -1.0, F32)
        self.segments = []
        off = 0
        for bq in range(nb):
            for u in range(nu):
                g = int(self.ntiles[bq, u])
                if g == 0:
                    continue
                self.segments.append(
                    {"u": u, "blk": bq, "ntiles": g, "start_edge": off})
                for c in range(NCORES):
                    sb, db = per[c][bq][u]
                    n = len(sb)
                    s_adj = sb - (HI_BASE if u == 1 else 0)
                    self.src_idx[c, off : off + n] = s_adj.astype(np.int16)
                    self.dstrel[c, off : off + n] = (db - 128 * bq).astype(F32)
                off += g * 128
        assert off == etot

    def core_arrays(self, c):
        dr = self.dstrel[c]
        i = np.nonzero(dr >= 0)[0]
        lane = i % 128
        tb = (i // 128) * 128
        d = dr[i].astype(np.int64)
        oh = np.zeros((128, self.etot), ml_dtypes.float8_e4m3)
        oh[lane, tb + d] = 1.0
        ohT = np.zeros((128, self.etot), ml_dtypes.float8_e4m3)
        ohT[d, tb + lane] = 1.0
        return _wrap_idx(self.src_idx[c]), oh, ohT

    def count_arrays(self, c, n_pad):
        cnt = self.counts[c].astype(F32)
        rcnt = 1.0 / np.maximum(cnt, 1.0)
        ind = (cnt > 0).astype(F32)
        rcnt_p = np.zeros(n_pad, F32)
        rcnt_p[: len(rcnt)] = rcnt
        ind_p = np.zeros(n_pad, F32)
        ind_p[: len(ind)] = ind
        return (
            rcnt_p[None, :].astype(BF16).copy(),
            ind_p[None, :].astype(BF16).copy(),
        )


def host_prep(inputs):
    p = {}
    cons_x = np.asarray(inputs["cons_x"], F32)
    var_x = np.asarray(inputs["var_x"], F32)
    edge_cons = np.asarray(inputs["edge_cons"]).astype(np.int64)
    edge_var = np.asarray(inputs["edge_var"]).astype(np.int64)
    head_mask = np.asarray(inputs["head_mask"]).astype(bool)

    NC, CF = cons_x.shape
    NV, VF = var_x.shape
    assert NC % NCORES == 0 and NV % NCORES == 0
    NCL, NVL = NC // NCORES, NV // NCORES
    p.update(NC=NC, NV=NV, CF=CF, VF=VF, NCL=NCL, NVL=NVL)

    p["conv1"] = ConvPrep(edge_cons, edge_var, NC, NV, NCL)
    p["conv2"] = ConvPrep(edge_var, edge_cons, NV, NC, NVL)

    NCLp = -(-NCL // 128) * 128
    NVLp = -(-NVL // 128) * 128
    NCLg = -(-NCLp // 512) * 512
    NVLg = -(-NVLp // 512) * 512
    p.update(NCLp=NCLp, NVLp=NVLp, NCLg=NCLg, NVLg=NVLg)

    w = {}

    def embed_w(prefix, g, b, w1, b1, w2, b2, feat):
        g = np.asarray(g, F32)
        b = np.asarray(b, F32)
        w1 = np.asarray(w1, F32)
        w1g = g[:, None] * w1
        aug = np.concatenate([w1g, np.zeros((1, w1.shape[1]), F32)], 0)
        w[prefix + "w1aug"] = aug.astype(BF16)
        w[prefix + "negw1bar"] = (-w1g.sum(0))[None, :].astype(BF16)
        w[prefix + "s1"] = _col(b @ w1 + np.asarray(b1, F32))
        w[prefix + "w2"] = np.asarray(w2, F32).astype(BF16)
        w[prefix + "b2"] = _col(np.asarray(b2, F32))
        mv = np.zeros((feat + 1, 1), F32)
        mv[:feat, 0] = 1.0 / feat
        w[prefix + "meanvec"] = mv.astype(BF16)

    embed_w("ce_", inputs["ce_ln_g"], inputs["ce_ln_b"], inputs["ce_w1"],
            inputs["ce_b1"], inputs["ce_w2"], inputs["ce_b2"], CF)
    embed_w("ve_", inputs["ve_ln_g"], inputs["ve_ln_b"], inputs["ve_w1"],
            inputs["ve_b1"], inputs["ve_w2"], inputs["ve_b2"], VF)

    for pre in ("vc_", "cv_"):
        wl = np.asarray(inputs[pre + "wl"], F32)
        w[pre + "wl"] = wl.astype(BF16)
        w[pre + "bl_row"] = _bcast_row(np.asarray(inputs[pre + "bl"], F32))
        w[pre + "wr"] = np.asarray(inputs[pre + "wr"], F32).astype(BF16)
        wf = np.asarray(inputs[pre + "wf"], F32)
        flg = np.asarray(inputs[pre + "flg"], F32)
        flb = np.asarray(inputs[pre + "flb"], F32)
        p[pre + "fl_trivial"] = bool(np.all(flg == 1.0) and np.all(flb == 0.0))
        w[pre + "wf"] = wf.astype(BF16)
        w[pre + "wfbar"] = (wf.sum(1) / EMB)[:, None].astype(BF16)
        bf = np.asarray(inputs[pre + "bf"], F32)
        w[pre + "bf_row"] = bf[None, :].astype(BF16)
        w[pre + "bfbar1"] = np.full((1, 1), bf.sum() / EMB, BF16)
        wo1 = np.asarray(inputs[pre + "wo1"], F32)
        plg = np.asarray(inputs[pre + "plg"], F32)
        plb = np.asarray(inputs[pre + "plb"], F32)
        wo1a = plg[:, None] * wo1[:EMB]
        w[pre + "wo1a"] = wo1a.astype(BF16)
        w[pre + "wo1abar"] = wo1a.sum(0)[None, :].astype(BF16)
        w[pre + "wo1b"] = wo1[EMB:].astype(BF16)
        w[pre + "bo1"] = _col(np.asarray(inputs[pre + "bo1"], F32) + plb @ wo1[:EMB])
        w[pre + "wo2"] = np.asarray(inputs[pre + "wo2"], F32).astype(BF16)
        w[pre + "bo2"] = _col(np.asarray(inputs[pre + "bo2"], F32))

    active = np.nonzero(head_mask)[0]
    nact = int(len(active))
    p["nact"] = nact
    denom = max(float(head_mask.sum()), 1.0)
    hb2 = np.asarray(inputs["hb2"], F32)
    p["out_scale"] = 1.0 / denom
    p["out_add"] = float(hb2[active].sum() / denom)
    if nact > 0:
        hw1 = np.asarray(inputs["hw1"], F32)[active]
        w["hw1"] = hw1.transpose(1, 0, 2).astype(BF16).copy()
        w["hb1"] = np.asarray(inputs["hb1"], F32)[active].T.copy()
        w["hw2"] = np.asarray(inputs["hw2"], F32)[active].T.astype(BF16).copy()

    w["ones_row"] = np.ones((1, 128), BF16)
    w["invemb_col"] = np.full((128, 1), 1.0 / EMB, BF16)
    p["weights"] = w

    core_inputs = []
    for c in range(NCORES):
        m = {}
        cx = cons_x[c * NCL : (c + 1) * NCL]
        vx = var_x[c * NVL : (c + 1) * NVL]
        cxp = np.zeros((NCLp, CF), F32)
        cxp[:NCL] = cx
        vxp = np.zeros((NVLp, VF), F32)
        vxp[:NVL] = vx
        m["consT_aug"] = np.concatenate([cxp.T, np.ones((1, NCLp), F32)], 0).astype(BF16)
        m["varT_aug"] = np.concatenate([vxp.T, np.ones((1, NVLp), F32)], 0).astype(BF16)
        s1, oh1, ohT1 = p["conv1"].core_arrays(c)
        m["e1_src"], m["e1_oh"], m["e1_ohT"] = s1, oh1, ohT1
        s2, oh2, ohT2 = p["conv2"].core_arrays(c)
        m["e2_src"], m["e2_oh"], m["e2_ohT"] = s2, oh2, ohT2
        m["rcnt1"], m["ind1"] = p["conv1"].count_arrays(c, NCLg)
        m["rcnt2"], m["ind2"] = p["conv2"].count_arrays(c, NVLg)
        for k, v in w.items():
            m[k] = v
        core_inputs.append(m)
    p["core_inputs"] = core_inputs
    return p


# ---------------------------------------------------------------------------


class B:
    def __init__(self, p):
        self.p = p
        self.nc = bacc.Bacc("TRN2", target_bir_lowering=False, debug=False,
                            num_devices=NCORES)
        self.d = {}

    def dram(self, name, shape, dtype, kind=None, addr_space=None):
        kw = {}
        if kind:
            kw["kind"] = kind
        if addr_space:
            kw["addr_space"] = addr_space
        t = self.nc.dram_tensor(name, list(shape), dtype, **kw)
        self.d[name] = t
        return t


def build_program(p):
    b = B(p)
    nc = b.nc
    w = p["weights"]
    NCL, NVL, NCLp, NVLp = p["NCL"], p["NVL"], p["NCLp"], p["NVLp"]
    NCLg, NVLg = p["NCLg"], p["NVLg"]
    CF, VF = p["CF"], p["VF"]
    NC, NV = p["NC"], p["NV"]
    nact = p["nact"]

    din = lambda n, s, t: b.dram(n, s, t, kind="ExternalInput")
    din("consT_aug", [CF + 1, NCLp], dt.bfloat16)
    din("varT_aug", [VF + 1, NVLp], dt.bfloat16)
    c1p, c2p = p["conv1"], p["conv2"]
    din("e1_src", [128, c1p.etot // 16], dt.int16)
    din("e1_oh", [128, c1p.etot], dt.float8e4)
    din("e1_ohT", [128, c1p.etot], dt.float8e4)
    din("e2_src", [128, c2p.etot // 16], dt.int16)
    din("e2_oh", [128, c2p.etot], dt.float8e4)
    din("e2_ohT", [128, c2p.etot], dt.float8e4)
    din("rcnt1", [1, NCLg], dt.bfloat16)
    din("ind1", [1, NCLg], dt.bfloat16)
    din("rcnt2", [1, NVLg], dt.bfloat16)
    din("ind2", [1, NVLg], dt.bfloat16)
    for k, v in w.items():
        din(k, list(v.shape), dt.bfloat16 if v.dtype == BF16 else dt.float32)
    out_d = b.dram("out", [1, NVLg], dt.float32, kind="ExternalOutput")

    lp1_loc = b.dram("lp1_loc", [NVL, EMB], dt.bfloat16)
    lp1_full = b.dram("lp1_full", [NV, EMB], dt.bfloat16, addr_space="Shared")
    rp1_loc = b.dram("rp1_loc", [NCL, EMB], dt.bfloat16)
    lp2_loc = b.dram("lp2_loc", [NCL, EMB], dt.bfloat16)
    lp2_full = b.dram("lp2_full", [NC, EMB], dt.bfloat16, addr_space="Shared")
    rp2_loc = b.dram("rp2_loc", [NVL, EMB], dt.bfloat16)

    KSTAGE = os.environ.get("KSTAGE", "full")

    with tile.TileContext(nc) as tc:
        nc.gpsimd.load_library(library_config.mlp)
        with (
            tc.tile_pool(name="const", bufs=1) as cpool,
            tc.tile_pool(name="resident", bufs=1) as rpool,
            tc.tile_pool(name="work", bufs=2) as wpool,
            tc.tile_pool(name="tiny", bufs=2) as tpool,
            tc.tile_pool(name="gath", bufs=2) as gpool,
            tc.tile_pool(name="psx", bufs=2, space="PSUM") as psx_pool,
            tc.tile_pool(name="agg", bufs=2, space="PSUM") as agg_pool,
            tc.tile_pool(name="mm", bufs=2, space="PSUM") as mm_pool,
            tc.tile_pool(name="bc", bufs=2, space="PSUM") as bc_pool,
        ):
            cw = {}
            for k, v in w.items():
                dtt = dt.bfloat16 if v.dtype == BF16 else dt.float32
                t = cpool.tile(list(v.shape), dtt, tag=k)
                nc.sync.dma_start(t[:], b.d[k][:])
                cw[k] = t
            for k, shp in (("rcnt1", [1, NCLg]), ("ind1", [1, NCLg]),
                           ("rcnt2", [1, NVLg]), ("ind2", [1, NVLg])):
                t = cpool.tile(shp, dt.bfloat16, tag=k)
                nc.sync.dma_start(t[:], b.d[k][:])
                cw[k] = t

            c0T = rpool.tile([128, NCLg], dt.bfloat16, tag="c0T")
            v0T = rpool.tile([128, NVLg], dt.bfloat16, tag="v0T")
            c1T = rpool.tile([128, NCLg], dt.bfloat16, tag="c1T")
            v1T = rpool.tile([128, NVLg], dt.bfloat16, tag="v1T")
            nc.vector.memset(c0T[:], 0.0)
            nc.vector.memset(v0T[:], 0.0)
            outrow = rpool.tile([1, NVLg], dt.float32, tag="outrow")
            eps_col = rpool.tile([128, 1], dt.float32, tag="eps_col")
            nc.vector.memset(eps_col[:], EPS)

            def rowmath_rstd(mu_row, ssq_row):
                """mu/ssq [1,512] f32 rows -> (rstd bf16, -mu*rstd bf16,
                mu bf16) rows [1,512]."""
                nmusq = tpool.tile([1, 512], dt.float32, tag="rm_nmusq")
                nc.vector.scalar_tensor_tensor(
                    nmusq[:], mu_row, -1.0, mu_row, AL.mult, AL.mult)
                veps = tpool.tile([1, 512], dt.float32, tag="rm_veps")
                nc.vector.scalar_tensor_tensor(
                    veps[:], ssq_row, EPS, nmusq[:], AL.add, AL.add)
                rstd = tpool.tile([1, 512], dt.float32, tag="rm_rstd")
                nc.scalar.activation(rstd[:], veps[:], ARS)
                rstd_bf = tpool.tile([1, 512], dt.bfloat16, tag="rm_rstdbf")
                nc.vector.tensor_copy(rstd_bf[:], rstd[:])
                nmur = tpool.tile([1, 512], dt.bfloat16, tag="rm_nmur")
                nc.vector.scalar_tensor_tensor(
                    nmur[:], mu_row, -1.0, rstd[:], AL.mult, AL.mult)
                mu_bf = tpool.tile([1, 512], dt.bfloat16, tag="rm_mubf")
                nc.vector.tensor_copy(mu_bf[:], mu_row)
                return rstd_bf, nmur, mu_bf

            # =========== embeds (feature-major, 512-col groups) ===========
            def embed(pre, xT_name, nfeat, ncols, ncols_g, outT, projs):
                xT = rpool.tile([nfeat + 1, ncols], dt.bfloat16, tag=pre + "xT")
                nc.sync.dma_start(xT[:], b.d[xT_name][:])
                for gi in range(ncols_g // 512):
                    cn = min(512, ncols - gi * 512)
                    if cn <= 0:
                        break
                    sl = slice(gi * 512, gi * 512 + cn)
                    xsq = wpool.tile([nfeat + 1, 512], dt.bfloat16, tag="exsq")
                    nc.vector.tensor_tensor(xsq[:, :cn], xT[:, sl], xT[:, sl],
                                            AL.mult)
                    pst = bc_pool.tile([128, 512], dt.float32, tag="bc")
                    nc.tensor.matmul(pst[0:1, :cn], cw[pre + "meanvec"][:],
                                     xT[:, sl], start=True, stop=True)
                    pst2 = bc_pool.tile([128, 512], dt.float32, tag="bc")
                    nc.tensor.matmul(pst2[0:1, :cn], cw[pre + "meanvec"][:],
                                     xsq[:, :cn], start=True, stop=True)
                    strow = tpool.tile([1, 512], dt.float32, tag="strow")
                    nc.vector.tensor_copy(strow[0:1, :cn], pst[0:1, :cn])
                    strow2 = tpool.tile([1, 512], dt.float32, tag="strow2")
                    nc.vector.tensor_copy(strow2[0:1, :cn], pst2[0:1, :cn])
                    rstd_bf, _, mu_bf = rowmath_rstd(strow[0:1, :], strow2[0:1, :])
                    psA = mm_pool.tile([128, 512], dt.float32, tag="mm")
                    nc.tensor.matmul(psA[:, :cn], cw[pre + "w1aug"][:], xT[:, sl],
                                     start=True, stop=False)
                    nc.tensor.matmul(psA[:, :cn], cw[pre + "negw1bar"][:],
                                     mu_bf[:, :cn], start=False, stop=True)
                    rb = bc_pool.tile([128, 512], dt.float32, tag="bc")
                    nc.tensor.matmul(rb[:, :cn], cw["ones_row"][:],
                                     rstd_bf[:, :cn], start=True, stop=True)
                    psA_sb = wpool.tile([128, 512], dt.bfloat16, tag="epsAsb")
                    nc.vector.tensor_copy(psA_sb[:, :cn], psA[:, :cn])
                    tmid = wpool.tile([128, 512], dt.bfloat16, tag="etmid")
                    nc.vector.tensor_tensor(tmid[:, :cn], psA_sb[:, :cn],
                                            rb[:, :cn], AL.mult)
                    z1 = wpool.tile([128, 512], dt.bfloat16, tag="ez1")
                    nc.scalar.activation(z1[:, :cn], tmid[:, :cn], LR,
                                         bias=cw[pre + "s1"][:], alpha=SLOPE)
                    psB = mm_pool.tile([128, 512], dt.float32, tag="mm")
                    nc.tensor.matmul(psB[:, :cn], cw[pre + "w2"][:], z1[:, :cn],
                                     start=True, stop=True)
                    nc.scalar.activation(outT[:, sl], psB[:, :cn], LR,
                                         bias=cw[pre + "b2"][:], alpha=SLOPE)
                    for (wname, brow, dout, n_valid) in projs:
                        for bi in range(4):
                            lo = gi * 512 + bi * 128
                            nv = min(128, max(0, n_valid - lo))
                            if nv == 0:
                                continue
                            psP = mm_pool.tile([128, 512], dt.float32, tag="mm")
                            nc.tensor.matmul(psP[:, :128], outT[:, lo : lo + 128],
                                             cw[wname][:], start=True, stop=True)
                            ob = wpool.tile([128, EMB], dt.bfloat16, tag="eob")
                            if brow is not None:
                                nc.vector.tensor_tensor(ob[:], psP[:, :128],
                                                        cw[brow][:], AL.add)
                            else:
                                nc.vector.tensor_copy(ob[:], psP[:, :128])
                            nc.sync.dma_start(b.d[dout][lo : lo + nv, :], ob[:nv, :])

            embed("ve_", "varT_aug", VF, NVLp, NVLg, v0T,
                  [("vc_wl", "vc_bl_row", "lp1_loc", NVL),
                   ("cv_wr", None, "rp2_loc", NVL)])
            if KSTAGE != "A":
                nc.gpsimd.collective_compute(
                    "AllGather", AL.bypass, ins=[lp1_loc[:]], outs=[lp1_full[:]],
                    replica_groups=[list(range(NCORES))])
            embed("ce_", "consT_aug", CF, NCLp, NCLg, c0T,
                  [("vc_wr", None, "rp1_loc", NCL)])

            # =========== conv (edges + interleaved feature-major post) =====
            def conv(cv, pre, lp_dram, rp_dram, src_d, oh_d, ohT_d, rightT, outT,
                     rcnt_name, ind_name, projs, n_valid, post_cb=None):
                nblocks = cv.nblocks
                ngroups = -(-nblocks // 4)
                grp_ps = [None] * ngroups
                grp_done = [0] * ngroups

                def post_group(g):
                    lo = g * 512
                    sl = slice(lo, lo + 512)
                    psG = grp_ps[g]
                    mean = wpool.tile([128, 512], dt.bfloat16, tag="pmean")
                    if psG is None:
                        nc.vector.memset(mean[:], 0.0)
                    else:
                        rcb = bc_pool.tile([128, 512], dt.float32, tag="bc")
                        nc.tensor.matmul(rcb[:], cw["ones_row"][:],
                                         cw[rcnt_name][:, sl], start=True,
                                         stop=True)
                        acc_sb = wpool.tile([128, 512], dt.bfloat16, tag="paccsb")
                        nc.vector.tensor_copy(acc_sb[:], psG[:])
                        grp_ps[g] = None
                        nc.vector.tensor_tensor(mean[:], acc_sb[:], rcb[:],
                                                AL.mult)
                    psU = mm_pool.tile([128, 512], dt.float32, tag="mm")
                    nc.tensor.matmul(psU[:], cw[pre + "wf"][:], mean[:],
                                     start=True, stop=False)
                    nc.tensor.matmul(psU[:], cw[pre + "bf_row"][:],
                                     cw[ind_name][:, sl], start=False, stop=True)
                    pst = bc_pool.tile([128, 512], dt.float32, tag="bc")
                    nc.tensor.matmul(pst[0:1, :], cw[pre + "wfbar"][:], mean[:],
                                     start=True, stop=False)
                    nc.tensor.matmul(pst[0:1, :], cw[pre + "bfbar1"][:],
                                     cw[ind_name][:, sl], start=False, stop=True)
                    u_sb = wpool.tile([128, 512], dt.bfloat16, tag="pusb")
                    nc.vector.tensor_copy(u_sb[:], psU[:])
                    usq = wpool.tile([128, 512], dt.bfloat16, tag="pmean")
                    nc.vector.tensor_tensor(usq[:], u_sb[:], u_sb[:], AL.mult)
                    pst2 = bc_pool.tile([128, 512], dt.float32, tag="bc")
                    nc.tensor.matmul(pst2[0:1, :], cw["invemb_col"][:], usq[:],
                                     start=True, stop=True)
                    strow = tpool.tile([1, 512], dt.float32, tag="strow")
                    nc.vector.tensor_copy(strow[0:1, :], pst[0:1, :])
                    strow2 = tpool.tile([1, 512], dt.float32, tag="strow2")
                    nc.vector.tensor_copy(strow2[0:1, :], pst2[0:1, :])
                    rstd_bf, nmur, _ = rowmath_rstd(strow[0:1, :], strow2[0:1, :])
                    rb = bc_pool.tile([128, 512], dt.float32, tag="bc")
                    nc.tensor.matmul(rb[:], cw["ones_row"][:], rstd_bf[:],
                                     start=True, stop=True)
                    t1 = wpool.tile([128, 512], dt.bfloat16, tag="pt1")
                    nc.vector.tensor_tensor(t1[:], u_sb[:], rb[:], AL.mult)
                    psB = mm_pool.tile([128, 512], dt.float32, tag="mm")
                    nc.tensor.matmul(psB[:], cw[pre + "wo1a"][:], t1[:],
                                     start=True, stop=False)
                    nc.tensor.matmul(psB[:], cw[pre + "wo1abar"][:], nmur[:],
                                     start=False, stop=False)
                    nc.tensor.matmul(psB[:], cw[pre + "wo1b"][:], rightT[:, sl],
                                     start=False, stop=True)
                    h2 = wpool.tile([128, 512], dt.bfloat16, tag="ph2")
                    nc.scalar.activation(h2[:], psB[:], LR, bias=cw[pre + "bo1"][:],
                                         alpha=SLOPE)
                    psC = mm_pool.tile([128, 512], dt.float32, tag="mm")
                    nc.tensor.matmul(psC[:], cw[pre + "wo2"][:], h2[:],
                                     start=True, stop=True)
                    nc.vector.tensor_scalar(outT[:, sl], psC[:], 1.0,
                                            cw[pre + "bo2"][:], AL.mult, AL.add)
                    for (wname, brow, dout) in projs:
                        for bi in range(4):
                            blo = lo + bi * 128
                            nv = min(128, max(0, n_valid - blo))
                            if nv == 0:
                                continue
                            psP = mm_pool.tile([128, 512], dt.float32, tag="mm")
                            nc.tensor.matmul(psP[:, :128], outT[:, blo : blo + 128],
                                             cw[wname][:], start=True, stop=True)
                            ob = wpool.tile([128, EMB], dt.bfloat16, tag="pob")
                            if brow is not None:
                                nc.vector.tensor_tensor(ob[:], psP[:, :128],
                                                        cw[brow][:], AL.add)
                            else:
                                nc.vector.tensor_copy(ob[:], psP[:, :128])
                            nc.sync.dma_start(b.d[dout][blo : blo + nv, :],
                                              ob[:nv, :])
                    if post_cb is not None:
                        post_cb(g)

                cur_rp = [None, -1]

                def get_rp(blk):
                    if cur_rp[1] == blk:
                        return cur_rp[0]
                    rp_sb = wpool.tile([128, EMB], dt.bfloat16, tag="rpblk")
                    lo = blk * 128
                    nv = min(128, n_valid - lo)
                    if nv < 128:
                        nc.vector.memset(rp_sb[:], 0.0)
                    nc.sync.dma_start(rp_sb[:nv, :], rp_dram[lo : lo + nv, :])
                    cur_rp[0] = rp_sb
                    cur_rp[1] = blk
                    return rp_sb

                for si, seg in enumerate(cv.segments):
                    blk = seg["blk"]
                    g = blk // 4
                    seg_first = (si == 0 or cv.segments[si - 1]["blk"] != blk)
                    seg_last = (si + 1 == len(cv.segments)
                                or cv.segments[si + 1]["blk"] != blk)
                    base_edge = seg["start_edge"]
                    ntiles = seg["ntiles"]
                    view_lo = HI_BASE if seg["u"] == 1 else 0
                    lp_view = lp_dram[view_lo:, :] if view_lo else lp_dram[:, :]
                    rp_sb = get_rp(blk)

                    tdone = 0
                    while tdone < ntiles:
                        tcn = min(CHUNK_TILES, ntiles - tdone)
                        e0 = base_edge + tdone * 128
                        ne = tcn * 128
                        sidx = gpool.tile([128, CHUNK_TILES * 8], dt.int16,
                                          tag="sidx")
                        nc.sync.dma_start(sidx[:, : ne // 16],
                                          src_d[:, e0 // 16 : (e0 + ne) // 16])
                        gbuf = gpool.tile([128, CHUNK_TILES, EMB], dt.bfloat16,
                                          tag="sgat")
                        nc.gpsimd.dma_gather(gbuf[:, :tcn, :], lp_view,
                                             sidx[:, : ne // 16], ne, ne, EMB,
                                             single_packet=False)
                        ohe = gpool.tile([128, CHUNK_TILES * 128], dt.float8e4,
                                         tag="ohe")
                        nc.sync.dma_start(ohe[:, :ne], oh_d[:, e0 : e0 + ne])
                        ohT = gpool.tile([128, CHUNK_TILES * 128], dt.float8e4,
                                         tag="ohT")
                        nc.sync.dma_start(ohT[:, :ne], ohT_d[:, e0 : e0 + ne])

                        xw_c = gpool.tile([128, CHUNK_TILES, EMB], dt.bfloat16,
                                          tag="xwc")
                        st6 = tpool.tile([128, CHUNK_TILES, 6], dt.float32,
                                         tag="st6")
                        mv_c = tpool.tile([128, CHUNK_TILES, 2], dt.float32,
                                          tag="mvc")

                        ngrp4 = -(-tcn // 4)
                        for g4 in range(ngrp4):
                            lo4 = g4 * 4
                            n4 = min(4, tcn - lo4)
                            psx = psx_pool.tile([128, 512], dt.float32, tag="psx")
                            for i in range(n4):
                                ti = lo4 + i
                                nc.tensor.matmul(
                                    psx[:, i * 128 : (i + 1) * 128],
                                    ohT[:, ti * 128 : (ti + 1) * 128], rp_sb[:],
                                    start=True, stop=True)
                            nc.vector.tensor_tensor(
                                xw_c[:, lo4 : lo4 + n4, :],
                                psx[:, : n4 * 128],
                                gbuf[:, lo4 : lo4 + n4, :], AL.add)
                            for i in range(n4):
                                ti = lo4 + i
                                nc.vector.bn_stats(st6[:, ti, :], xw_c[:, ti, :])
                                nc.vector.bn_aggr(mv_c[:, ti, :], st6[:, ti, :])

                        rstd_t = tpool.tile([128, CHUNK_TILES], dt.float32,
                                            tag="rstdc")
                        nc.scalar.activation(rstd_t[:, :tcn], mv_c[:, :tcn, 1],
                                             ARS, bias=eps_col[:])
                        nmr_c = tpool.tile([128, CHUNK_TILES], dt.float32,
                                           tag="nmrc")
                        nc.vector.scalar_tensor_tensor(
                            nmr_c[:, :tcn], mv_c[:, :tcn, 0], -1.0,
                            rstd_t[:, :tcn], AL.mult, AL.mult)

                        for ti in range(tcn):
                            act = wpool.tile([128, EMB], dt.bfloat16, tag="act")
                            nc.scalar.activation(
                                act[:], xw_c[:, ti, :], LR,
                                bias=nmr_c[:, ti : ti + 1],
                                scale=rstd_t[:, ti : ti + 1], alpha=SLOPE)
                            if grp_ps[g] is None:
                                agg_t = agg_pool.tile([128, 512], dt.float32,
                                                      tag="agg")
                                grp_ps[g] = agg_t
                            first = seg_first and tdone == 0 and ti == 0
                            last = seg_last and (tdone + ti + 1 == ntiles)
                            bslot = blk % 4
                            nc.tensor.matmul(
                                grp_ps[g][:, bslot * 128 : (bslot + 1) * 128],
                                act[:], ohe[:, ti * 128 : (ti + 1) * 128],
                                start=first, stop=last)
                        tdone += tcn

                    if seg_last:
                        grp_done[g] += 1
                        gnb = min(4, nblocks - g * 4)
                        if grp_done[g] == gnb:
                            post_group(g)

                for g in range(ngroups):
                    gnb = min(4, nblocks - g * 4)
                    if grp_done[g] < gnb:
                        post_group(g)

            # =========== heads ===========
            def heads_chunk(j):
                sl = slice(j * 512, (j + 1) * 512)
                if nact == 0:
                    nc.vector.memset(outrow[:, sl], 0.0)
                    return
                pso = bc_pool.tile([128, 512], dt.float32, tag="bc")
                for hi in range(nact):
                    psH = mm_pool.tile([128, 512], dt.float32, tag="mm")
                    nc.tensor.matmul(psH[:], cw["hw1"][:, hi, :], v1T[:, sl],
                                     start=True, stop=True)
                    hh = wpool.tile([128, 512], dt.bfloat16, tag="hh")
                    nc.scalar.activation(hh[:], psH[:], LR,
                                         bias=cw["hb1"][:, hi : hi + 1],
                                         alpha=SLOPE)
                    nc.tensor.matmul(pso[0:1, :], cw["hw2"][:, hi : hi + 1], hh[:],
                                     start=(hi == 0), stop=(hi == nact - 1))
                nc.vector.tensor_scalar(outrow[:, sl], pso[0:1, :],
                                        p["out_scale"], p["out_add"],
                                        AL.mult, AL.add)

            # =========== run ===========
            if KSTAGE == "A":
                nc.vector.memset(outrow[:], 0.0)
            else:
                conv(c1p, "vc_", lp1_full, rp1_loc, b.d["e1_src"],
                     b.d["e1_oh"], b.d["e1_ohT"], c0T, c1T, "rcnt1", "ind1",
                     [("cv_wl", "cv_bl_row", "lp2_loc")], NCL)
                if KSTAGE == "C1":
                    nc.vector.memset(outrow[:], 0.0)
                else:
                    nc.gpsimd.collective_compute(
                        "AllGather", AL.bypass, ins=[lp2_loc[:]],
                        outs=[lp2_full[:]], replica_groups=[list(range(NCORES))])
                    heads_done = set()

                    def post2_cb(g):
                        if g not in heads_done:
                            heads_done.add(g)
                            heads_chunk(g)

                    conv(c2p, "cv_", lp2_full, rp2_loc, b.d["e2_src"],
                         b.d["e2_oh"], b.d["e2_ohT"], v0T, v1T, "rcnt2", "ind2",
                         [], NVL, post_cb=post2_cb)
                    for j in range(NVLg // 512):
                        if j not in heads_done:
                            heads_chunk(j)
            nc.sync.dma_start(out_d[:], outrow[:])

    nc.compile()
    return b


_CACHE = {}


def kernel(**inputs):
    key = tuple(sorted((k, tuple(np.asarray(v).shape)) for k, v in inputs.items()))
    p = host_prep(inputs)
    ck = (key, p["nact"], p["conv1"].etot, p["conv2"].etot)
    if ck in _CACHE:
        b = _CACHE[ck]
    else:
        b = build_program(p)
        _CACHE[ck] = b
    in_maps = [dict(p["core_inputs"][c]) for c in range(NCORES)]
    res = run_bass_kernel_spmd(b.nc, in_maps, core_ids=list(range(NCORES)))
    NVL = p["NVL"]
    out = np.concatenate([res.results[c]["out"][0, :NVL] for c in range(NCORES)])
    return out.astype(np.float32)


# revision 17
# speedup vs baseline: 1.0278x; 1.0278x over previous
"""Trainium2 Bass kernel for BipartiteGCN (8 NeuronCores, SPMD) — v2.

Strategy:
 - Node rows sharded 8 ways; edges sharded by DESTINATION range, grouped by
   (dst_block, src_bucket) segments with src-sorted order inside.
 - Edge loop (edge-major [e, f] tiles of 128 edges):
   dma_gather lp[src] rows (bf16), one-hot matmul broadcasts rp[dst] into
   PSUM, one DVE add per 4 tiles, grouped bn_stats, ScalarE fused
   (x-mu)*rstd + LeakyReLU, acc^T matmul (feature-major segment sum,
   4 dst-blocks per PSUM bank).
 - Scatter-mean denominators precomputed on host (no ones column).
 - Post-conv + embed MLPs run feature-major: batched N=512 matmuls with
   stationary weights, LN via colsum matmuls + row math, rank-1 bias /
   mean-correction matmuls, biases as per-partition columns. No PE
   transposes anywhere.
 - lp tables all-gathered (bf16); heads interleaved with conv2 post.
"""

import os
import sys

for _p in ("/opt/trn_rl_repo",):
    if _p not in sys.path:
        sys.path.insert(0, _p)

import numpy as np
import ml_dtypes

import concourse.bass as bass
import concourse.bacc as bacc
import concourse.mybir as mybir
from concourse import tile, library_config
from concourse.bass_utils import run_bass_kernel_spmd

BF16 = ml_dtypes.bfloat16
F32 = np.float32
NCORES = 8
EMB = 128
CHUNK_TILES = 16
HI_BASE = 32768
EPS = 1e-5
SLOPE = 0.01

dt = mybir.dt
AL = mybir.AluOpType
LR = mybir.ActivationFunctionType.Lrelu
SQ = mybir.ActivationFunctionType.Sqrt
ARS = mybir.ActivationFunctionType.Abs_reciprocal_sqrt


def _wrap_idx(idx_i16):
    n = idx_i16.shape[0]
    assert n % 16 == 0
    w = idx_i16.reshape(n // 16, 16).T
    return np.tile(w, (8, 1)).copy()


def _col(v):
    return np.asarray(v, F32)[:, None].copy()


def _bcast_row(v, rows=128):
    return np.broadcast_to(np.asarray(v, F32)[None, :], (rows, v.shape[0])).copy()


class ConvPrep:
    """Per-conv edge data: segments of (dst_block, src_bucket), src-sorted."""

    def __init__(self, dst, src, n_dst, n_src, dst_per_core):
        self.n_dst_local = dst_per_core
        self.nblocks = -(-dst_per_core // 128)
        self.two_buckets = n_src > HI_BASE
        nb = self.nblocks
        nu = 2 if self.two_buckets else 1

        core = dst // dst_per_core
        dloc_all = dst - core * dst_per_core
        block_all = dloc_all // 128

        self.counts = np.zeros((NCORES, dst_per_core), np.int64)
        for c in range(NCORES):
            m = core == c
            self.counts[c] = np.bincount(dloc_all[m], minlength=dst_per_core)

        per = [[[None] * nu for _ in range(nb)] for _ in range(NCORES)]
        for c in range(NCORES):
            m = core == c
            d_c = dloc_all[m]
            s_c = src[m]
            b_c = block_all[m]
            u_c = (s_c >= HI_BASE).astype(np.int8) if self.two_buckets else np.zeros(
                len(s_c), np.int8
            )
            for bq in range(nb):
                for u in range(nu):
                    mu = (b_c == bq) & (u_c == u)
                    order = np.argsort(s_c[mu], kind="stable")
                    per[c][bq][u] = (s_c[mu][order], d_c[mu][order])

        self.ntiles = np.zeros((nb, nu), np.int64)
        for bq in range(nb):
            for u in range(nu):
                mx = max(len(per[c][bq][u][0]) for c in range(NCORES))
                self.ntiles[bq, u] = -(-mx // 128) if mx > 0 else 0

        etot = int(self.ntiles.sum()) * 128
        self.etot = etot

        self.src_idx = np.zeros((NCORES, etot), np.int16)
        self.dstrel = np.full((NCORES, etot), -1.0, F32)
        self.segments = []
        off = 0
        for bq in range(nb):
            for u in range(nu):
                g = int(self.ntiles[bq, u])
                if g == 0:
                    continue
                self.segments.append(
                    {"u": u, "blk": bq, "ntiles": g, "start_edge": off})
                for c in range(NCORES):
                    sb, db = per[c][bq][u]
                    n = len(sb)
                    s_adj = sb - (HI_BASE if u == 1 else 0)
                    self.src_idx[c, off : off + n] = s_adj.astype(np.int16)
                    self.dstrel[c, off : off + n] = (db - 128 * bq).astype(F32)
                off += g * 128
        assert off == etot

    def core_arrays(self, c):
        dr = self.dstrel[c]
        i = np.nonzero(dr >= 0)[0]
        lane = i % 128
        tb = (i // 128) * 128
        d = dr[i].astype(np.int64)
        oh = np.zeros((128, self.etot), ml_dtypes.float8_e4m3)
        oh[lane, tb + d] = 1.0
        ohT = np.zeros((128, self.etot), ml_dtypes.float8_e4m3)
        ohT[d, tb + lane] = 1.0
        return _wrap_idx(self.src_idx[c]), oh, ohT

    def count_arrays(self, c, n_pad):
        cnt = self.counts[c].astype(F32)
        rcnt = 1.0 / np.maximum(cnt, 1.0)
        ind = (cnt > 0).astype(F32)
        rcnt_p = np.zeros(n_pad, F32)
        rcnt_p[: len(rcnt)] = rcnt
        ind_p = np.zeros(n_pad, F32)
        ind_p[: len(ind)] = ind
        return (
            rcnt_p[None, :].astype(BF16).copy(),
            ind_p[None, :].astype(BF16).copy(),
        )


def host_prep(inputs):
    p = {}
    cons_x = np.asarray(inputs["cons_x"], F32)
    var_x = np.asarray(inputs["var_x"], F32)
    edge_cons = np.asarray(inputs["edge_cons"]).astype(np.int64)
    edge_var = np.asarray(inputs["edge_var"]).astype(np.int64)
    head_mask = np.asarray(inputs["head_mask"]).astype(bool)

    NC, CF = cons_x.shape
    NV, VF = var_x.shape
    assert NC % NCORES == 0 and NV % NCORES == 0
    NCL, NVL = NC // NCORES, NV // NCORES
    p.update(NC=NC, NV=NV, CF=CF, VF=VF, NCL=NCL, NVL=NVL)

    p["conv1"] = ConvPrep(edge_cons, edge_var, NC, NV, NCL)
    p["conv2"] = ConvPrep(edge_var, edge_cons, NV, NC, NVL)

    NCLp = -(-NCL // 128) * 128
    NVLp = -(-NVL // 128) * 128
    NCLg = -(-NCLp // 512) * 512
    NVLg = -(-NVLp // 512) * 512
    p.update(NCLp=NCLp, NVLp=NVLp, NCLg=NCLg, NVLg=NVLg)

    w = {}

    def embed_w(prefix, g, b, w1, b1, w2, b2, feat):
        g = np.asarray(g, F32)
        b = np.asarray(b, F32)
        w1 = np.asarray(w1, F32)
        w1g = g[:, None] * w1
        aug = np.concatenate([w1g, np.zeros((1, w1.shape[1]), F32)], 0)
        w[prefix + "w1aug"] = aug.astype(BF16)
        w[prefix + "negw1bar"] = (-w1g.sum(0))[None, :].astype(BF16)
        w[prefix + "s1"] = _col(b @ w1 + np.asarray(b1, F32))
        w[prefix + "w2"] = np.asarray(w2, F32).astype(BF16)
        w[prefix + "b2"] = _col(np.asarray(b2, F32))
        mv = np.zeros((feat + 1, 1), F32)
        mv[:feat, 0] = 1.0 / feat
        w[prefix + "meanvec"] = mv.astype(BF16)

    embed_w("ce_", inputs["ce_ln_g"], inputs["ce_ln_b"], inputs["ce_w1"],
            inputs["ce_b1"], inputs["ce_w2"], inputs["ce_b2"], CF)
    embed_w("ve_", inputs["ve_ln_g"], inputs["ve_ln_b"], inputs["ve_w1"],
            inputs["ve_b1"], inputs["ve_w2"], inputs["ve_b2"], VF)

    for pre in ("vc_", "cv_"):
        wl = np.asarray(inputs[pre + "wl"], F32)
        w[pre + "wl"] = wl.astype(BF16)
        w[pre + "bl_row"] = _bcast_row(np.asarray(inputs[pre + "bl"], F32))
        w[pre + "wr"] = np.asarray(inputs[pre + "wr"], F32).astype(BF16)
        wf = np.asarray(inputs[pre + "wf"], F32)
        flg = np.asarray(inputs[pre + "flg"], F32)
        flb = np.asarray(inputs[pre + "flb"], F32)
        p[pre + "fl_trivial"] = bool(np.all(flg == 1.0) and np.all(flb == 0.0))
        w[pre + "wf"] = wf.astype(BF16)
        w[pre + "wfbar"] = (wf.sum(1) / EMB)[:, None].astype(BF16)
        bf = np.asarray(inputs[pre + "bf"], F32)
        w[pre + "bf_row"] = bf[None, :].astype(BF16)
        w[pre + "bfbar1"] = np.full((1, 1), bf.sum() / EMB, BF16)
        wo1 = np.asarray(inputs[pre + "wo1"], F32)
        plg = np.asarray(inputs[pre + "plg"], F32)
        plb = np.asarray(inputs[pre + "plb"], F32)
        wo1a = plg[:, None] * wo1[:EMB]
        w[pre + "wo1a"] = wo1a.astype(BF16)
        w[pre + "wo1abar"] = wo1a.sum(0)[None, :].astype(BF16)
        w[pre + "wo1b"] = wo1[EMB:].astype(BF16)
        w[pre + "bo1"] = _col(np.asarray(inputs[pre + "bo1"], F32) + plb @ wo1[:EMB])
        w[pre + "wo2"] = np.asarray(inputs[pre + "wo2"], F32).astype(BF16)
        w[pre + "bo2"] = _col(np.asarray(inputs[pre + "bo2"], F32))

    active = np.nonzero(head_mask)[0]
    nact = int(len(active))
    p["nact"] = nact
    denom = max(float(head_mask.sum()), 1.0)
    hb2 = np.asarray(inputs["hb2"], F32)
    p["out_scale"] = 1.0 / denom
    p["out_add"] = float(hb2[active].sum() / denom)
    if nact > 0:
        hw1 = np.asarray(inputs["hw1"], F32)[active]
        w["hw1"] = hw1.transpose(1, 0, 2).astype(BF16).copy()
        w["hb1"] = np.asarray(inputs["hb1"], F32)[active].T.copy()
        w["hw2"] = np.asarray(inputs["hw2"], F32)[active].T.astype(BF16).copy()

    w["ones_row"] = np.ones((1, 128), BF16)
    w["invemb_col"] = np.full((128, 1), 1.0 / EMB, BF16)
    p["weights"] = w

    core_inputs = []
    for c in range(NCORES):
        m = {}
        cx = cons_x[c * NCL : (c + 1) * NCL]
        vx = var_x[c * NVL : (c + 1) * NVL]
        cxp = np.zeros((NCLp, CF), F32)
        cxp[:NCL] = cx
        vxp = np.zeros((NVLp, VF), F32)
        vxp[:NVL] = vx
        m["consT_aug"] = np.concatenate([cxp.T, np.ones((1, NCLp), F32)], 0).astype(BF16)
        m["varT_aug"] = np.concatenate([vxp.T, np.ones((1, NVLp), F32)], 0).astype(BF16)
        s1, oh1, ohT1 = p["conv1"].core_arrays(c)
        m["e1_src"], m["e1_oh"], m["e1_ohT"] = s1, oh1, ohT1
        s2, oh2, ohT2 = p["conv2"].core_arrays(c)
        m["e2_src"], m["e2_oh"], m["e2_ohT"] = s2, oh2, ohT2
        m["rcnt1"], m["ind1"] = p["conv1"].count_arrays(c, NCLg)
        m["rcnt2"], m["ind2"] = p["conv2"].count_arrays(c, NVLg)
        for k, v in w.items():
            m[k] = v
        core_inputs.append(m)
    p["core_inputs"] = core_inputs
    return p


# ---------------------------------------------------------------------------


class B:
    def __init__(self, p):
        self.p = p
        self.nc = bacc.Bacc("TRN2", target_bir_lowering=False, debug=False,
                            num_devices=NCORES)
        self.d = {}

    def dram(self, name, shape, dtype, kind=None, addr_space=None):
        kw = {}
        if kind:
            kw["kind"] = kind
        if addr_space:
            kw["addr_space"] = addr_space
        t = self.nc.dram_tensor(name, list(shape), dtype, **kw)
        self.d[name] = t
        return t


def build_program(p):
    b = B(p)
    nc = b.nc
    w = p["weights"]
    NCL, NVL, NCLp, NVLp = p["NCL"], p["NVL"], p["NCLp"], p["NVLp"]
    NCLg, NVLg = p["NCLg"], p["NVLg"]
    CF, VF = p["CF"], p["VF"]
    NC, NV = p["NC"], p["NV"]
    nact = p["nact"]

    din = lambda n, s, t: b.dram(n, s, t, kind="ExternalInput")
    din("consT_aug", [CF + 1, NCLp], dt.bfloat16)
    din("varT_aug", [VF + 1, NVLp], dt.bfloat16)
    c1p, c2p = p["conv1"], p["conv2"]
    din("e1_src", [128, c1p.etot // 16], dt.int16)
    din("e1_oh", [128, c1p.etot], dt.float8e4)
    din("e1_ohT", [128, c1p.etot], dt.float8e4)
    din("e2_src", [128, c2p.etot // 16], dt.int16)
    din("e2_oh", [128, c2p.etot], dt.float8e4)
    din("e2_ohT", [128, c2p.etot], dt.float8e4)
    din("rcnt1", [1, NCLg], dt.bfloat16)
    din("ind1", [1, NCLg], dt.bfloat16)
    din("rcnt2", [1, NVLg], dt.bfloat16)
    din("ind2", [1, NVLg], dt.bfloat16)
    for k, v in w.items():
        din(k, list(v.shape), dt.bfloat16 if v.dtype == BF16 else dt.float32)
    out_d = b.dram("out", [1, NVLg], dt.float32, kind="ExternalOutput")

    lp1_loc = b.dram("lp1_loc", [NVL, EMB], dt.bfloat16)
    lp1_full = b.dram("lp1_full", [NV, EMB], dt.bfloat16, addr_space="Shared")
    rp1_loc = b.dram("rp1_loc", [NCL, EMB], dt.bfloat16)
    lp2_loc = b.dram("lp2_loc", [NCL, EMB], dt.bfloat16)
    lp2_full = b.dram("lp2_full", [NC, EMB], dt.bfloat16, addr_space="Shared")
    rp2_loc = b.dram("rp2_loc", [NVL, EMB], dt.bfloat16)

    KSTAGE = os.environ.get("KSTAGE", "full")

    with tile.TileContext(nc) as tc:
        nc.gpsimd.load_library(library_config.mlp)
        with (
            tc.tile_pool(name="const", bufs=1) as cpool,
            tc.tile_pool(name="resident", bufs=1) as rpool,
            tc.tile_pool(name="work", bufs=2) as wpool,
            tc.tile_pool(name="tiny", bufs=2) as tpool,
            tc.tile_pool(name="gath", bufs=2) as gpool,
            tc.tile_pool(name="psx", bufs=2, space="PSUM") as psx_pool,
            tc.tile_pool(name="agg", bufs=2, space="PSUM") as agg_pool,
            tc.tile_pool(name="mm", bufs=2, space="PSUM") as mm_pool,
            tc.tile_pool(name="bc", bufs=2, space="PSUM") as bc_pool,
        ):
            cw = {}
            for k, v in w.items():
                dtt = dt.bfloat16 if v.dtype == BF16 else dt.float32
                t = cpool.tile(list(v.shape), dtt, tag=k)
                nc.sync.dma_start(t[:], b.d[k][:])
                cw[k] = t
            for k, shp in (("rcnt1", [1, NCLg]), ("ind1", [1, NCLg]),
                           ("rcnt2", [1, NVLg]), ("ind2", [1, NVLg])):
                t = cpool.tile(shp, dt.bfloat16, tag=k)
                nc.sync.dma_start(t[:], b.d[k][:])
                cw[k] = t

            c0T = rpool.tile([128, NCLg], dt.bfloat16, tag="c0T")
            v0T = rpool.tile([128, NVLg], dt.bfloat16, tag="v0T")
            c1T = rpool.tile([128, NCLg], dt.bfloat16, tag="c1T")
            v1T = rpool.tile([128, NVLg], dt.bfloat16, tag="v1T")
            nc.vector.memset(c0T[:], 0.0)
            nc.vector.memset(v0T[:], 0.0)
            outrow = rpool.tile([1, NVLg], dt.float32, tag="outrow")

            def rowmath_rstd(mu_row, ssq_row):
                """mu/ssq [1,512] f32 rows -> (rstd bf16, -mu*rstd bf16,
                mu bf16) rows [1,512]."""
                nmusq = tpool.tile([1, 512], dt.float32, tag="rm_nmusq")
                nc.vector.scalar_tensor_tensor(
                    nmusq[:], mu_row, -1.0, mu_row, AL.mult, AL.mult)
                veps = tpool.tile([1, 512], dt.float32, tag="rm_veps")
                nc.vector.scalar_tensor_tensor(
                    veps[:], ssq_row, EPS, nmusq[:], AL.add, AL.add)
                sd = tpool.tile([1, 512], dt.float32, tag="rm_sd")
                nc.scalar.activation(sd[:], veps[:], SQ)
                rstd = tpool.tile([1, 512], dt.float32, tag="rm_rstd")
                nc.vector.reciprocal(rstd[:], sd[:])
                rstd_bf = tpool.tile([1, 512], dt.bfloat16, tag="rm_rstdbf")
                nc.vector.tensor_copy(rstd_bf[:], rstd[:])
                nmur = tpool.tile([1, 512], dt.bfloat16, tag="rm_nmur")
                nc.vector.scalar_tensor_tensor(
                    nmur[:], mu_row, -1.0, rstd[:], AL.mult, AL.mult)
                mu_bf = tpool.tile([1, 512], dt.bfloat16, tag="rm_mubf")
                nc.vector.tensor_copy(mu_bf[:], mu_row)
                return rstd_bf, nmur, mu_bf

            # =========== embeds (feature-major, 512-col groups) ===========
            def embed(pre, xT_name, nfeat, ncols, ncols_g, outT, projs):
                xT = rpool.tile([nfeat + 1, ncols], dt.bfloat16, tag=pre + "xT")
                nc.sync.dma_start(xT[:], b.d[xT_name][:])
                for gi in range(ncols_g // 512):
                    cn = min(512, ncols - gi * 512)
                    if cn <= 0:
                        break
                    sl = slice(gi * 512, gi * 512 + cn)
                    xsq = wpool.tile([nfeat + 1, 512], dt.bfloat16, tag="exsq")
                    nc.vector.tensor_tensor(xsq[:, :cn], xT[:, sl], xT[:, sl],
                                            AL.mult)
                    pst = bc_pool.tile([128, 512], dt.float32, tag="bc")
                    nc.tensor.matmul(pst[0:1, :cn], cw[pre + "meanvec"][:],
                                     xT[:, sl], start=True, stop=True)
                    pst2 = bc_pool.tile([128, 512], dt.float32, tag="bc")
                    nc.tensor.matmul(pst2[0:1, :cn], cw[pre + "meanvec"][:],
                                     xsq[:, :cn], start=True, stop=True)
                    strow = tpool.tile([1, 512], dt.float32, tag="strow")
                    nc.vector.tensor_copy(strow[0:1, :cn], pst[0:1, :cn])
                    strow2 = tpool.tile([1, 512], dt.float32, tag="strow2")
                    nc.vector.tensor_copy(strow2[0:1, :cn], pst2[0:1, :cn])
                    rstd_bf, _, mu_bf = rowmath_rstd(strow[0:1, :], strow2[0:1, :])
                    psA = mm_pool.tile([128, 512], dt.float32, tag="mm")
                    nc.tensor.matmul(psA[:, :cn], cw[pre + "w1aug"][:], xT[:, sl],
                                     start=True, stop=False)
                    nc.tensor.matmul(psA[:, :cn], cw[pre + "negw1bar"][:],
                                     mu_bf[:, :cn], start=False, stop=True)
                    rb = bc_pool.tile([128, 512], dt.float32, tag="bc")
                    nc.tensor.matmul(rb[:, :cn], cw["ones_row"][:],
                                     rstd_bf[:, :cn], start=True, stop=True)
                    psA_sb = wpool.tile([128, 512], dt.bfloat16, tag="epsAsb")
                    nc.vector.tensor_copy(psA_sb[:, :cn], psA[:, :cn])
                    tmid = wpool.tile([128, 512], dt.bfloat16, tag="etmid")
                    nc.vector.tensor_tensor(tmid[:, :cn], psA_sb[:, :cn],
                                            rb[:, :cn], AL.mult)
                    z1 = wpool.tile([128, 512], dt.bfloat16, tag="ez1")
                    nc.scalar.activation(z1[:, :cn], tmid[:, :cn], LR,
                                         bias=cw[pre + "s1"][:], alpha=SLOPE)
                    psB = mm_pool.tile([128, 512], dt.float32, tag="mm")
                    nc.tensor.matmul(psB[:, :cn], cw[pre + "w2"][:], z1[:, :cn],
                                     start=True, stop=True)
                    nc.scalar.activation(outT[:, sl], psB[:, :cn], LR,
                                         bias=cw[pre + "b2"][:], alpha=SLOPE)
                    for (wname, brow, dout, n_valid) in projs:
                        for bi in range(4):
                            lo = gi * 512 + bi * 128
                            nv = min(128, max(0, n_valid - lo))
                            if nv == 0:
                                continue
                            psP = mm_pool.tile([128, 512], dt.float32, tag="mm")
                            nc.tensor.matmul(psP[:, :128], outT[:, lo : lo + 128],
                                             cw[wname][:], start=True, stop=True)
                            ob = wpool.tile([128, EMB], dt.bfloat16, tag="eob")
                            if brow is not None:
                                nc.vector.tensor_tensor(ob[:], psP[:, :128],
                                                        cw[brow][:], AL.add)
                            else:
                                nc.vector.tensor_copy(ob[:], psP[:, :128])
                            nc.sync.dma_start(b.d[dout][lo : lo + nv, :], ob[:nv, :])

            embed("ve_", "varT_aug", VF, NVLp, NVLg, v0T,
                  [("vc_wl", "vc_bl_row", "lp1_loc", NVL),
                   ("cv_wr", None, "rp2_loc", NVL)])
            if KSTAGE != "A":
                nc.gpsimd.collective_compute(
                    "AllGather", AL.bypass, ins=[lp1_loc[:]], outs=[lp1_full[:]],
                    replica_groups=[list(range(NCORES))])
            embed("ce_", "consT_aug", CF, NCLp, NCLg, c0T,
                  [("vc_wr", None, "rp1_loc", NCL)])

            # =========== conv (edges + interleaved feature-major post) =====
            def conv(cv, pre, lp_dram, rp_dram, src_d, oh_d, ohT_d, rightT, outT,
                     rcnt_name, ind_name, projs, n_valid, post_cb=None):
                nblocks = cv.nblocks
                ngroups = -(-nblocks // 4)
                grp_ps = [None] * ngroups
                grp_done = [0] * ngroups

                def post_group(g):
                    lo = g * 512
                    sl = slice(lo, lo + 512)
                    psG = grp_ps[g]
                    mean = wpool.tile([128, 512], dt.bfloat16, tag="pmean")
                    if psG is None:
                        nc.vector.memset(mean[:], 0.0)
                    else:
                        rcb = bc_pool.tile([128, 512], dt.float32, tag="bc")
                        nc.tensor.matmul(rcb[:], cw["ones_row"][:],
                                         cw[rcnt_name][:, sl], start=True,
                                         stop=True)
                        acc_sb = wpool.tile([128, 512], dt.bfloat16, tag="paccsb")
                        nc.vector.tensor_copy(acc_sb[:], psG[:])
                        grp_ps[g] = None
                        nc.vector.tensor_tensor(mean[:], acc_sb[:], rcb[:],
                                                AL.mult)
                    psU = mm_pool.tile([128, 512], dt.float32, tag="mm")
                    nc.tensor.matmul(psU[:], cw[pre + "wf"][:], mean[:],
                                     start=True, stop=False)
                    nc.tensor.matmul(psU[:], cw[pre + "bf_row"][:],
                                     cw[ind_name][:, sl], start=False, stop=True)
                    pst = bc_pool.tile([128, 512], dt.float32, tag="bc")
                    nc.tensor.matmul(pst[0:1, :], cw[pre + "wfbar"][:], mean[:],
                                     start=True, stop=False)
                    nc.tensor.matmul(pst[0:1, :], cw[pre + "bfbar1"][:],
                                     cw[ind_name][:, sl], start=False, stop=True)
                    u_sb = wpool.tile([128, 512], dt.bfloat16, tag="pusb")
                    nc.vector.tensor_copy(u_sb[:], psU[:])
                    usq = wpool.tile([128, 512], dt.bfloat16, tag="pmean")
                    nc.vector.tensor_tensor(usq[:], u_sb[:], u_sb[:], AL.mult)
                    pst2 = bc_pool.tile([128, 512], dt.float32, tag="bc")
                    nc.tensor.matmul(pst2[0:1, :], cw["invemb_col"][:], usq[:],
                                     start=True, stop=True)
                    strow = tpool.tile([1, 512], dt.float32, tag="strow")
                    nc.vector.tensor_copy(strow[0:1, :], pst[0:1, :])
                    strow2 = tpool.tile([1, 512], dt.float32, tag="strow2")
                    nc.vector.tensor_copy(strow2[0:1, :], pst2[0:1, :])
                    rstd_bf, nmur, _ = rowmath_rstd(strow[0:1, :], strow2[0:1, :])
                    rb = bc_pool.tile([128, 512], dt.float32, tag="bc")
                    nc.tensor.matmul(rb[:], cw["ones_row"][:], rstd_bf[:],
                                     start=True, stop=True)
                    t1 = wpool.tile([128, 512], dt.bfloat16, tag="pt1")
                    nc.vector.tensor_tensor(t1[:], u_sb[:], rb[:], AL.mult)
                    psB = mm_pool.tile([128, 512], dt.float32, tag="mm")
                    nc.tensor.matmul(psB[:], cw[pre + "wo1a"][:], t1[:],
                                     start=True, stop=False)
                    nc.tensor.matmul(psB[:], cw[pre + "wo1abar"][:], nmur[:],
                                     start=False, stop=False)
                    nc.tensor.matmul(psB[:], cw[pre + "wo1b"][:], rightT[:, sl],
                                     start=False, stop=True)
                    h2 = wpool.tile([128, 512], dt.bfloat16, tag="ph2")
                    nc.scalar.activation(h2[:], psB[:], LR, bias=cw[pre + "bo1"][:],
                                         alpha=SLOPE)
                    psC = mm_pool.tile([128, 512], dt.float32, tag="mm")
                    nc.tensor.matmul(psC[:], cw[pre + "wo2"][:], h2[:],
                                     start=True, stop=True)
                    nc.vector.tensor_scalar(outT[:, sl], psC[:], 1.0,
                                            cw[pre + "bo2"][:], AL.mult, AL.add)
                    for (wname, brow, dout) in projs:
                        for bi in range(4):
                            blo = lo + bi * 128
                            nv = min(128, max(0, n_valid - blo))
                            if nv == 0:
                                continue
                            psP = mm_pool.tile([128, 512], dt.float32, tag="mm")
                            nc.tensor.matmul(psP[:, :128], outT[:, blo : blo + 128],
                                             cw[wname][:], start=True, stop=True)
                            ob = wpool.tile([128, EMB], dt.bfloat16, tag="pob")
                            if brow is not None:
                                nc.vector.tensor_tensor(ob[:], psP[:, :128],
                                                        cw[brow][:], AL.add)
                            else:
                                nc.vector.tensor_copy(ob[:], psP[:, :128])
                            nc.sync.dma_start(b.d[dout][blo : blo + nv, :],
                                              ob[:nv, :])
                    if post_cb is not None:
                        post_cb(g)

                cur_rp = [None, -1]

                def get_rp(blk):
                    if cur_rp[1] == blk:
                        return cur_rp[0]
                    rp_sb = wpool.tile([128, EMB], dt.bfloat16, tag="rpblk")
                    lo = blk * 128
                    nv = min(128, n_valid - lo)
                    if nv < 128:
                        nc.vector.memset(rp_sb[:], 0.0)
                    nc.sync.dma_start(rp_sb[:nv, :], rp_dram[lo : lo + nv, :])
                    cur_rp[0] = rp_sb
                    cur_rp[1] = blk
                    return rp_sb

                for si, seg in enumerate(cv.segments):
                    blk = seg["blk"]
                    g = blk // 4
                    seg_first = (si == 0 or cv.segments[si - 1]["blk"] != blk)
                    seg_last = (si + 1 == len(cv.segments)
                                or cv.segments[si + 1]["blk"] != blk)
                    base_edge = seg["start_edge"]
                    ntiles = seg["ntiles"]
                    view_lo = HI_BASE if seg["u"] == 1 else 0
                    lp_view = lp_dram[view_lo:, :] if view_lo else lp_dram[:, :]
                    rp_sb = get_rp(blk)

                    tdone = 0
                    while tdone < ntiles:
                        tcn = min(CHUNK_TILES, ntiles - tdone)
                        e0 = base_edge + tdone * 128
                        ne = tcn * 128
                        sidx = gpool.tile([128, CHUNK_TILES * 8], dt.int16,
                                          tag="sidx")
                        nc.sync.dma_start(sidx[:, : ne // 16],
                                          src_d[:, e0 // 16 : (e0 + ne) // 16])
                        gbuf = gpool.tile([128, CHUNK_TILES, EMB], dt.bfloat16,
                                          tag="sgat")
                        nc.gpsimd.dma_gather(gbuf[:, :tcn, :], lp_view,
                                             sidx[:, : ne // 16], ne, ne, EMB,
                                             single_packet=False)
                        ohe = gpool.tile([128, CHUNK_TILES * 128], dt.float8e4,
                                         tag="ohe")
                        nc.sync.dma_start(ohe[:, :ne], oh_d[:, e0 : e0 + ne])
                        ohT = gpool.tile([128, CHUNK_TILES * 128], dt.float8e4,
                                         tag="ohT")
                        nc.sync.dma_start(ohT[:, :ne], ohT_d[:, e0 : e0 + ne])

                        xw_c = gpool.tile([128, CHUNK_TILES, EMB], dt.bfloat16,
                                          tag="xwc")
                        st6 = tpool.tile([128, CHUNK_TILES, 6], dt.float32,
                                         tag="st6")
                        mv_c = tpool.tile([128, CHUNK_TILES, 2], dt.float32,
                                          tag="mvc")

                        ngrp4 = -(-tcn // 4)
                        for g4 in range(ngrp4):
                            lo4 = g4 * 4
                            n4 = min(4, tcn - lo4)
                            psx = psx_pool.tile([128, 512], dt.float32, tag="psx")
                            for i in range(n4):
                                ti = lo4 + i
                                nc.tensor.matmul(
                                    psx[:, i * 128 : (i + 1) * 128],
                                    ohT[:, ti * 128 : (ti + 1) * 128], rp_sb[:],
                                    start=True, stop=True)
                            nc.vector.tensor_tensor(
                                xw_c[:, lo4 : lo4 + n4, :],
                                psx[:, : n4 * 128],
                                gbuf[:, lo4 : lo4 + n4, :], AL.add)
                            for i in range(n4):
                                ti = lo4 + i
                                nc.vector.bn_stats(st6[:, ti, :], xw_c[:, ti, :])
                                nc.vector.bn_aggr(mv_c[:, ti, :], st6[:, ti, :])

                        veps = tpool.tile([128, CHUNK_TILES], dt.float32,
                                          tag="vepsc")
                        nc.vector.tensor_scalar(veps[:, :tcn], mv_c[:, :tcn, 1],
                                                EPS, None, AL.add)
                        sdc = tpool.tile([128, CHUNK_TILES], dt.float32, tag="sdc")
                        nc.scalar.activation(sdc[:, :tcn], veps[:, :tcn], SQ)
                        rstd_t = tpool.tile([128, CHUNK_TILES], dt.float32,
                                            tag="rstdc")
                        nc.vector.reciprocal(rstd_t[:, :tcn], sdc[:, :tcn])
                        nmr_c = tpool.tile([128, CHUNK_TILES], dt.float32,
                                           tag="nmrc")
                        nc.vector.scalar_tensor_tensor(
                            nmr_c[:, :tcn], mv_c[:, :tcn, 0], -1.0,
                            rstd_t[:, :tcn], AL.mult, AL.mult)

                        for ti in range(tcn):
                            act = wpool.tile([128, EMB], dt.bfloat16, tag="act")
                            nc.scalar.activation(
                                act[:], xw_c[:, ti, :], LR,
                                bias=nmr_c[:, ti : ti + 1],
                                scale=rstd_t[:, ti : ti + 1], alpha=SLOPE)
                            if grp_ps[g] is None:
                                agg_t = agg_pool.tile([128, 512], dt.float32,
                                                      tag="agg")
                                grp_ps[g] = agg_t
                            first = seg_first and tdone == 0 and ti == 0
                            last = seg_last and (tdone + ti + 1 == ntiles)
                            bslot = blk % 4
                            nc.tensor.matmul(
                                grp_ps[g][:, bslot * 128 : (bslot + 1) * 128],
                                act[:], ohe[:, ti * 128 : (ti + 1) * 128],
                                start=first, stop=last)
                        tdone += tcn

                    if seg_last:
                        grp_done[g] += 1
                        gnb = min(4, nblocks - g * 4)
                        if grp_done[g] == gnb:
                            post_group(g)

                for g in range(ngroups):
                    gnb = min(4, nblocks - g * 4)
                    if grp_done[g] < gnb:
                        post_group(g)

            # =========== heads ===========
            def heads_chunk(j):
                sl = slice(j * 512, (j + 1) * 512)
                if nact == 0:
                    nc.vector.memset(outrow[:, sl], 0.0)
                    return
                pso = bc_pool.tile([128, 512], dt.float32, tag="bc")
                for hi in range(nact):
                    psH = mm_pool.tile([128, 512], dt.float32, tag="mm")
                    nc.tensor.matmul(psH[:], cw["hw1"][:, hi, :], v1T[:, sl],
                                     start=True, stop=True)
                    hh = wpool.tile([128, 512], dt.bfloat16, tag="hh")
                    nc.scalar.activation(hh[:], psH[:], LR,
                                         bias=cw["hb1"][:, hi : hi + 1],
                                         alpha=SLOPE)
                    nc.tensor.matmul(pso[0:1, :], cw["hw2"][:, hi : hi + 1], hh[:],
                                     start=(hi == 0), stop=(hi == nact - 1))
                nc.vector.tensor_scalar(outrow[:, sl], pso[0:1, :],
                                        p["out_scale"], p["out_add"],
                                        AL.mult, AL.add)

            # =========== run ===========
            if KSTAGE == "A":
                nc.vector.memset(outrow[:], 0.0)
            else:
                conv(c1p, "vc_", lp1_full, rp1_loc, b.d["e1_src"],
                     b.d["e1_oh"], b.d["e1_ohT"], c0T, c1T, "rcnt1", "ind1",
                     [("cv_wl", "cv_bl_row", "lp2_loc")], NCL)
                if KSTAGE == "C1":
                    nc.vector.memset(outrow[:], 0.0)
                else:
                    nc.gpsimd.collective_compute(
                        "AllGather", AL.bypass, ins=[lp2_loc[:]],
                        outs=[lp2_full[:]], replica_groups=[list(range(NCORES))])
                    heads_done = set()

                    def post2_cb(g):
                        if g not in heads_done:
                            heads_done.add(g)
                            heads_chunk(g)

                    conv(c2p, "cv_", lp2_full, rp2_loc, b.d["e2_src"],
                         b.d["e2_oh"], b.d["e2_ohT"], v0T, v1T, "rcnt2", "ind2",
                         [], NVL, post_cb=post2_cb)
                    for j in range(NVLg // 512):
                        if j not in heads_done:
                            heads_chunk(j)
            nc.sync.dma_start(out_d[:], outrow[:])

    nc.compile()
    return b


_CACHE = {}


def kernel(**inputs):
    key = tuple(sorted((k, tuple(np.asarray(v).shape)) for k, v in inputs.items()))
    p = host_prep(inputs)
    ck = (key, p["nact"], p["conv1"].etot, p["conv2"].etot)
    if ck in _CACHE:
        b = _CACHE[ck]
    else:
        b = build_program(p)
        _CACHE[ck] = b
    in_maps = [dict(p["core_inputs"][c]) for c in range(NCORES)]
    res = run_bass_kernel_spmd(b.nc, in_maps, core_ids=list(range(NCORES)))
    NVL = p["NVL"]
    out = np.concatenate([res.results[c]["out"][0, :NVL] for c in range(NCORES)])
    return out.astype(np.float32)


# revision 18
# speedup vs baseline: 1.2568x; 1.2227x over previous
"""Trainium2 Bass kernel for BipartiteGCN (8 NeuronCores, SPMD) — v2.

Strategy:
 - Node rows sharded 8 ways; edges sharded by DESTINATION range, grouped by
   (dst_block, src_bucket) segments with src-sorted order inside.
 - Edge loop (edge-major [e, f] tiles of 128 edges):
   dma_gather lp[src] rows (bf16), one-hot matmul broadcasts rp[dst] into
   PSUM, one DVE add per 4 tiles, grouped bn_stats, ScalarE fused
   (x-mu)*rstd + LeakyReLU, acc^T matmul (feature-major segment sum,
   4 dst-blocks per PSUM bank).
 - Scatter-mean denominators precomputed on host (no ones column).
 - Post-conv + embed MLPs run feature-major: batched N=512 matmuls with
   stationary weights, LN via colsum matmuls + row math, rank-1 bias /
   mean-correction matmuls, biases as per-partition columns. No PE
   transposes anywhere.
 - lp tables all-gathered (bf16); heads interleaved with conv2 post.
"""

import os
import sys

for _p in ("/opt/trn_rl_repo",):
    if _p not in sys.path:
        sys.path.insert(0, _p)

import numpy as np
import ml_dtypes

import concourse.bass as bass
import concourse.bacc as bacc
import concourse.mybir as mybir
from concourse import tile, library_config
from concourse.bass_utils import run_bass_kernel_spmd

BF16 = ml_dtypes.bfloat16
F32 = np.float32
NCORES = 8
EMB = 128
CHUNK_TILES = 26
HI_BASE = 32768
EPS = 1e-5
SLOPE = 0.01

dt = mybir.dt
AL = mybir.AluOpType
LR = mybir.ActivationFunctionType.Lrelu
SQ = mybir.ActivationFunctionType.Sqrt
ARS = mybir.ActivationFunctionType.Abs_reciprocal_sqrt


def _wrap_idx(idx_i16):
    n = idx_i16.shape[0]
    assert n % 16 == 0
    w = idx_i16.reshape(n // 16, 16).T
    return np.tile(w, (8, 1)).copy()


def _col(v):
    return np.asarray(v, F32)[:, None].copy()


def _bcast_row(v, rows=128):
    return np.broadcast_to(np.asarray(v, F32)[None, :], (rows, v.shape[0])).copy()


class ConvPrep:
    """Per-conv edge data: segments of (dst_block, src_bucket), src-sorted."""

    def __init__(self, dst, src, n_dst, n_src, dst_per_core):
        self.n_dst_local = dst_per_core
        self.nblocks = -(-dst_per_core // 128)
        self.two_buckets = n_src > HI_BASE
        nb = self.nblocks
        nu = 2 if self.two_buckets else 1

        core = dst // dst_per_core
        dloc_all = dst - core * dst_per_core
        block_all = dloc_all // 128

        self.counts = np.zeros((NCORES, dst_per_core), np.int64)
        for c in range(NCORES):
            m = core == c
            self.counts[c] = np.bincount(dloc_all[m], minlength=dst_per_core)

        per = [[[None] * nu for _ in range(nb)] for _ in range(NCORES)]
        for c in range(NCORES):
            m = core == c
            d_c = dloc_all[m]
            s_c = src[m]
            b_c = block_all[m]
            u_c = (s_c >= HI_BASE).astype(np.int8) if self.two_buckets else np.zeros(
                len(s_c), np.int8
            )
            for bq in range(nb):
                for u in range(nu):
                    mu = (b_c == bq) & (u_c == u)
                    order = np.argsort(s_c[mu], kind="stable")
                    per[c][bq][u] = (s_c[mu][order], d_c[mu][order])

        self.ntiles = np.zeros((nb, nu), np.int64)
        for bq in range(nb):
            for u in range(nu):
                mx = max(len(per[c][bq][u][0]) for c in range(NCORES))
                self.ntiles[bq, u] = -(-mx // 128) if mx > 0 else 0

        etot = int(self.ntiles.sum()) * 128
        self.etot = etot

        self.src_idx = np.zeros((NCORES, etot), np.int16)
        self.dstrel = np.full((NCORES, etot), -1.0, F32)
        self.segments = []
        off = 0
        for bq in range(nb):
            for u in range(nu):
                g = int(self.ntiles[bq, u])
                if g == 0:
                    continue
                self.segments.append(
                    {"u": u, "blk": bq, "ntiles": g, "start_edge": off})
                for c in range(NCORES):
                    sb, db = per[c][bq][u]
                    n = len(sb)
                    s_adj = sb - (HI_BASE if u == 1 else 0)
                    self.src_idx[c, off : off + n] = s_adj.astype(np.int16)
                    self.dstrel[c, off : off + n] = (db - 128 * bq).astype(F32)
                off += g * 128
        assert off == etot

    def core_arrays(self, c):
        dr = self.dstrel[c]
        i = np.nonzero(dr >= 0)[0]
        lane = i % 128
        tb = (i // 128) * 128
        d = dr[i].astype(np.int64)
        oh = np.zeros((128, self.etot), ml_dtypes.float8_e4m3)
        oh[lane, tb + d] = 1.0
        ohT = np.zeros((128, self.etot), ml_dtypes.float8_e4m3)
        ohT[d, tb + lane] = 1.0
        return _wrap_idx(self.src_idx[c]), oh, ohT

    def count_arrays(self, c, n_pad):
        cnt = self.counts[c].astype(F32)
        rcnt = 1.0 / np.maximum(cnt, 1.0)
        ind = (cnt > 0).astype(F32)
        rcnt_p = np.zeros(n_pad, F32)
        rcnt_p[: len(rcnt)] = rcnt
        ind_p = np.zeros(n_pad, F32)
        ind_p[: len(ind)] = ind
        return (
            rcnt_p[None, :].astype(BF16).copy(),
            ind_p[None, :].astype(BF16).copy(),
        )


def host_prep(inputs):
    p = {}
    cons_x = np.asarray(inputs["cons_x"], F32)
    var_x = np.asarray(inputs["var_x"], F32)
    edge_cons = np.asarray(inputs["edge_cons"]).astype(np.int64)
    edge_var = np.asarray(inputs["edge_var"]).astype(np.int64)
    head_mask = np.asarray(inputs["head_mask"]).astype(bool)

    NC, CF = cons_x.shape
    NV, VF = var_x.shape
    assert NC % NCORES == 0 and NV % NCORES == 0
    NCL, NVL = NC // NCORES, NV // NCORES
    p.update(NC=NC, NV=NV, CF=CF, VF=VF, NCL=NCL, NVL=NVL)

    p["conv1"] = ConvPrep(edge_cons, edge_var, NC, NV, NCL)
    p["conv2"] = ConvPrep(edge_var, edge_cons, NV, NC, NVL)

    NCLp = -(-NCL // 128) * 128
    NVLp = -(-NVL // 128) * 128
    NCLg = -(-NCLp // 512) * 512
    NVLg = -(-NVLp // 512) * 512
    p.update(NCLp=NCLp, NVLp=NVLp, NCLg=NCLg, NVLg=NVLg)

    w = {}

    def embed_w(prefix, g, b, w1, b1, w2, b2, feat):
        g = np.asarray(g, F32)
        b = np.asarray(b, F32)
        w1 = np.asarray(w1, F32)
        w1g = g[:, None] * w1
        aug = np.concatenate([w1g, np.zeros((1, w1.shape[1]), F32)], 0)
        w[prefix + "w1aug"] = aug.astype(BF16)
        w[prefix + "negw1bar"] = (-w1g.sum(0))[None, :].astype(BF16)
        w[prefix + "s1"] = _col(b @ w1 + np.asarray(b1, F32))
        w[prefix + "w2"] = np.asarray(w2, F32).astype(BF16)
        w[prefix + "b2"] = _col(np.asarray(b2, F32))
        mv = np.zeros((feat + 1, 1), F32)
        mv[:feat, 0] = 1.0 / feat
        w[prefix + "meanvec"] = mv.astype(BF16)

    embed_w("ce_", inputs["ce_ln_g"], inputs["ce_ln_b"], inputs["ce_w1"],
            inputs["ce_b1"], inputs["ce_w2"], inputs["ce_b2"], CF)
    embed_w("ve_", inputs["ve_ln_g"], inputs["ve_ln_b"], inputs["ve_w1"],
            inputs["ve_b1"], inputs["ve_w2"], inputs["ve_b2"], VF)

    for pre in ("vc_", "cv_"):
        wl = np.asarray(inputs[pre + "wl"], F32)
        w[pre + "wl"] = wl.astype(BF16)
        w[pre + "bl_row"] = _bcast_row(np.asarray(inputs[pre + "bl"], F32))
        w[pre + "wr"] = np.asarray(inputs[pre + "wr"], F32).astype(BF16)
        wf = np.asarray(inputs[pre + "wf"], F32)
        flg = np.asarray(inputs[pre + "flg"], F32)
        flb = np.asarray(inputs[pre + "flb"], F32)
        p[pre + "fl_trivial"] = bool(np.all(flg == 1.0) and np.all(flb == 0.0))
        w[pre + "wf"] = wf.astype(BF16)
        w[pre + "wfbar"] = (wf.sum(1) / EMB)[:, None].astype(BF16)
        bf = np.asarray(inputs[pre + "bf"], F32)
        w[pre + "bf_row"] = bf[None, :].astype(BF16)
        w[pre + "bfbar1"] = np.full((1, 1), bf.sum() / EMB, BF16)
        wo1 = np.asarray(inputs[pre + "wo1"], F32)
        plg = np.asarray(inputs[pre + "plg"], F32)
        plb = np.asarray(inputs[pre + "plb"], F32)
        wo1a = plg[:, None] * wo1[:EMB]
        w[pre + "wo1a"] = wo1a.astype(BF16)
        w[pre + "wo1abar"] = wo1a.sum(0)[None, :].astype(BF16)
        w[pre + "wo1b"] = wo1[EMB:].astype(BF16)
        w[pre + "bo1"] = _col(np.asarray(inputs[pre + "bo1"], F32) + plb @ wo1[:EMB])
        w[pre + "wo2"] = np.asarray(inputs[pre + "wo2"], F32).astype(BF16)
        w[pre + "bo2"] = _col(np.asarray(inputs[pre + "bo2"], F32))

    active = np.nonzero(head_mask)[0]
    nact = int(len(active))
    p["nact"] = nact
    denom = max(float(head_mask.sum()), 1.0)
    hb2 = np.asarray(inputs["hb2"], F32)
    p["out_scale"] = 1.0 / denom
    p["out_add"] = float(hb2[active].sum() / denom)
    if nact > 0:
        hw1 = np.asarray(inputs["hw1"], F32)[active]
        w["hw1"] = hw1.transpose(1, 0, 2).astype(BF16).copy()
        w["hb1"] = np.asarray(inputs["hb1"], F32)[active].T.copy()
        w["hw2"] = np.asarray(inputs["hw2"], F32)[active].T.astype(BF16).copy()

    w["ones_row"] = np.ones((1, 128), BF16)
    w["invemb_col"] = np.full((128, 1), 1.0 / EMB, BF16)
    p["weights"] = w

    core_inputs = []
    for c in range(NCORES):
        m = {}
        cx = cons_x[c * NCL : (c + 1) * NCL]
        vx = var_x[c * NVL : (c + 1) * NVL]
        cxp = np.zeros((NCLp, CF), F32)
        cxp[:NCL] = cx
        vxp = np.zeros((NVLp, VF), F32)
        vxp[:NVL] = vx
        m["consT_aug"] = np.concatenate([cxp.T, np.ones((1, NCLp), F32)], 0).astype(BF16)
        m["varT_aug"] = np.concatenate([vxp.T, np.ones((1, NVLp), F32)], 0).astype(BF16)
        s1, oh1, ohT1 = p["conv1"].core_arrays(c)
        m["e1_src"], m["e1_oh"], m["e1_ohT"] = s1, oh1, ohT1
        s2, oh2, ohT2 = p["conv2"].core_arrays(c)
        m["e2_src"], m["e2_oh"], m["e2_ohT"] = s2, oh2, ohT2
        m["rcnt1"], m["ind1"] = p["conv1"].count_arrays(c, NCLg)
        m["rcnt2"], m["ind2"] = p["conv2"].count_arrays(c, NVLg)
        for k, v in w.items():
            m[k] = v
        core_inputs.append(m)
    p["core_inputs"] = core_inputs
    return p


# ---------------------------------------------------------------------------


class B:
    def __init__(self, p):
        self.p = p
        self.nc = bacc.Bacc("TRN2", target_bir_lowering=False, debug=False,
                            num_devices=NCORES)
        self.d = {}

    def dram(self, name, shape, dtype, kind=None, addr_space=None):
        kw = {}
        if kind:
            kw["kind"] = kind
        if addr_space:
            kw["addr_space"] = addr_space
        t = self.nc.dram_tensor(name, list(shape), dtype, **kw)
        self.d[name] = t
        return t


def build_program(p):
    b = B(p)
    nc = b.nc
    w = p["weights"]
    NCL, NVL, NCLp, NVLp = p["NCL"], p["NVL"], p["NCLp"], p["NVLp"]
    NCLg, NVLg = p["NCLg"], p["NVLg"]
    CF, VF = p["CF"], p["VF"]
    NC, NV = p["NC"], p["NV"]
    nact = p["nact"]

    din = lambda n, s, t: b.dram(n, s, t, kind="ExternalInput")
    din("consT_aug", [CF + 1, NCLp], dt.bfloat16)
    din("varT_aug", [VF + 1, NVLp], dt.bfloat16)
    c1p, c2p = p["conv1"], p["conv2"]
    din("e1_src", [128, c1p.etot // 16], dt.int16)
    din("e1_oh", [128, c1p.etot], dt.float8e4)
    din("e1_ohT", [128, c1p.etot], dt.float8e4)
    din("e2_src", [128, c2p.etot // 16], dt.int16)
    din("e2_oh", [128, c2p.etot], dt.float8e4)
    din("e2_ohT", [128, c2p.etot], dt.float8e4)
    din("rcnt1", [1, NCLg], dt.bfloat16)
    din("ind1", [1, NCLg], dt.bfloat16)
    din("rcnt2", [1, NVLg], dt.bfloat16)
    din("ind2", [1, NVLg], dt.bfloat16)
    for k, v in w.items():
        din(k, list(v.shape), dt.bfloat16 if v.dtype == BF16 else dt.float32)
    out_d = b.dram("out", [1, NVLg], dt.float32, kind="ExternalOutput")

    lp1_loc = b.dram("lp1_loc", [NVL, EMB], dt.bfloat16)
    lp1_full = b.dram("lp1_full", [NV, EMB], dt.bfloat16, addr_space="Shared")
    rp1_loc = b.dram("rp1_loc", [NCL, EMB], dt.bfloat16)
    lp2_loc = b.dram("lp2_loc", [NCL, EMB], dt.bfloat16)
    lp2_full = b.dram("lp2_full", [NC, EMB], dt.bfloat16, addr_space="Shared")
    rp2_loc = b.dram("rp2_loc", [NVL, EMB], dt.bfloat16)

    KSTAGE = os.environ.get("KSTAGE", "full")

    with tile.TileContext(nc) as tc:
        nc.gpsimd.load_library(library_config.mlp)
        with (
            tc.tile_pool(name="const", bufs=1) as cpool,
            tc.tile_pool(name="resident", bufs=1) as rpool,
            tc.tile_pool(name="work", bufs=2) as wpool,
            tc.tile_pool(name="tiny", bufs=2) as tpool,
            tc.tile_pool(name="gath", bufs=2) as gpool,
            tc.tile_pool(name="psx", bufs=2, space="PSUM") as psx_pool,
            tc.tile_pool(name="agg", bufs=2, space="PSUM") as agg_pool,
            tc.tile_pool(name="mm", bufs=2, space="PSUM") as mm_pool,
            tc.tile_pool(name="bc", bufs=2, space="PSUM") as bc_pool,
        ):
            cw = {}
            for k, v in w.items():
                dtt = dt.bfloat16 if v.dtype == BF16 else dt.float32
                t = cpool.tile(list(v.shape), dtt, tag=k)
                nc.sync.dma_start(t[:], b.d[k][:])
                cw[k] = t
            for k, shp in (("rcnt1", [1, NCLg]), ("ind1", [1, NCLg]),
                           ("rcnt2", [1, NVLg]), ("ind2", [1, NVLg])):
                t = cpool.tile(shp, dt.bfloat16, tag=k)
                nc.sync.dma_start(t[:], b.d[k][:])
                cw[k] = t

            c0T = rpool.tile([128, NCLg], dt.bfloat16, tag="c0T")
            v0T = rpool.tile([128, NVLg], dt.bfloat16, tag="v0T")
            c1T = rpool.tile([128, NCLg], dt.bfloat16, tag="c1T")
            v1T = rpool.tile([128, NVLg], dt.bfloat16, tag="v1T")
            nc.vector.memset(c0T[:], 0.0)
            nc.vector.memset(v0T[:], 0.0)
            outrow = rpool.tile([1, NVLg], dt.float32, tag="outrow")

            def rowmath_rstd(mu_row, ssq_row):
                """mu/ssq [1,512] f32 rows -> (rstd bf16, -mu*rstd bf16,
                mu bf16) rows [1,512]."""
                nmusq = tpool.tile([1, 512], dt.float32, tag="rm_nmusq")
                nc.vector.scalar_tensor_tensor(
                    nmusq[:], mu_row, -1.0, mu_row, AL.mult, AL.mult)
                veps = tpool.tile([1, 512], dt.float32, tag="rm_veps")
                nc.vector.scalar_tensor_tensor(
                    veps[:], ssq_row, EPS, nmusq[:], AL.add, AL.add)
                sd = tpool.tile([1, 512], dt.float32, tag="rm_sd")
                nc.scalar.activation(sd[:], veps[:], SQ)
                rstd = tpool.tile([1, 512], dt.float32, tag="rm_rstd")
                nc.vector.reciprocal(rstd[:], sd[:])
                rstd_bf = tpool.tile([1, 512], dt.bfloat16, tag="rm_rstdbf")
                nc.vector.tensor_copy(rstd_bf[:], rstd[:])
                nmur = tpool.tile([1, 512], dt.bfloat16, tag="rm_nmur")
                nc.vector.scalar_tensor_tensor(
                    nmur[:], mu_row, -1.0, rstd[:], AL.mult, AL.mult)
                mu_bf = tpool.tile([1, 512], dt.bfloat16, tag="rm_mubf")
                nc.vector.tensor_copy(mu_bf[:], mu_row)
                return rstd_bf, nmur, mu_bf

            # =========== embeds (feature-major, 512-col groups) ===========
            def embed(pre, xT_name, nfeat, ncols, ncols_g, outT, projs):
                for gi in range(ncols_g // 512):
                    cn = min(512, ncols - gi * 512)
                    if cn <= 0:
                        break
                    sl = slice(gi * 512, gi * 512 + cn)
                    xT_g = wpool.tile([nfeat + 1, 512], dt.bfloat16, tag="exslice")
                    nc.sync.dma_start(xT_g[:, :cn],
                                      b.d[xT_name][:, gi * 512 : gi * 512 + cn])
                    xsq = wpool.tile([nfeat + 1, 512], dt.bfloat16, tag="exsq")
                    nc.vector.tensor_tensor(xsq[:, :cn], xT_g[:, :cn], xT_g[:, :cn],
                                            AL.mult)
                    pst = bc_pool.tile([128, 512], dt.float32, tag="bc")
                    nc.tensor.matmul(pst[0:1, :cn], cw[pre + "meanvec"][:],
                                     xT_g[:, :cn], start=True, stop=True)
                    pst2 = bc_pool.tile([128, 512], dt.float32, tag="bc")
                    nc.tensor.matmul(pst2[0:1, :cn], cw[pre + "meanvec"][:],
                                     xsq[:, :cn], start=True, stop=True)
                    strow = tpool.tile([1, 512], dt.float32, tag="strow")
                    nc.vector.tensor_copy(strow[0:1, :cn], pst[0:1, :cn])
                    strow2 = tpool.tile([1, 512], dt.float32, tag="strow2")
                    nc.vector.tensor_copy(strow2[0:1, :cn], pst2[0:1, :cn])
                    rstd_bf, _, mu_bf = rowmath_rstd(strow[0:1, :], strow2[0:1, :])
                    psA = mm_pool.tile([128, 512], dt.float32, tag="mm")
                    nc.tensor.matmul(psA[:, :cn], cw[pre + "w1aug"][:],
                                     xT_g[:, :cn], start=True, stop=False)
                    nc.tensor.matmul(psA[:, :cn], cw[pre + "negw1bar"][:],
                                     mu_bf[:, :cn], start=False, stop=True)
                    rb = bc_pool.tile([128, 512], dt.float32, tag="bc")
                    nc.tensor.matmul(rb[:, :cn], cw["ones_row"][:],
                                     rstd_bf[:, :cn], start=True, stop=True)
                    psA_sb = wpool.tile([128, 512], dt.bfloat16, tag="epsAsb")
                    nc.vector.tensor_copy(psA_sb[:, :cn], psA[:, :cn])
                    tmid = wpool.tile([128, 512], dt.bfloat16, tag="etmid")
                    nc.vector.tensor_tensor(tmid[:, :cn], psA_sb[:, :cn],
                                            rb[:, :cn], AL.mult)
                    z1 = wpool.tile([128, 512], dt.bfloat16, tag="ez1")
                    nc.scalar.activation(z1[:, :cn], tmid[:, :cn], LR,
                                         bias=cw[pre + "s1"][:], alpha=SLOPE)
                    psB = mm_pool.tile([128, 512], dt.float32, tag="mm")
                    nc.tensor.matmul(psB[:, :cn], cw[pre + "w2"][:], z1[:, :cn],
                                     start=True, stop=True)
                    nc.scalar.activation(outT[:, sl], psB[:, :cn], LR,
                                         bias=cw[pre + "b2"][:], alpha=SLOPE)
                    for (wname, brow, dout, n_valid) in projs:
                        for bi in range(4):
                            lo = gi * 512 + bi * 128
                            nv = min(128, max(0, n_valid - lo))
                            if nv == 0:
                                continue
                            psP = mm_pool.tile([128, 512], dt.float32, tag="mm")
                            nc.tensor.matmul(psP[:, :128], outT[:, lo : lo + 128],
                                             cw[wname][:], start=True, stop=True)
                            ob = wpool.tile([128, EMB], dt.bfloat16, tag="eob")
                            if brow is not None:
                                nc.vector.tensor_tensor(ob[:], psP[:, :128],
                                                        cw[brow][:], AL.add)
                            else:
                                nc.vector.tensor_copy(ob[:], psP[:, :128])
                            nc.sync.dma_start(b.d[dout][lo : lo + nv, :], ob[:nv, :])

            embed("ve_", "varT_aug", VF, NVLp, NVLg, v0T,
                  [("vc_wl", "vc_bl_row", "lp1_loc", NVL),
                   ("cv_wr", None, "rp2_loc", NVL)])
            if KSTAGE != "A":
                nc.gpsimd.collective_compute(
                    "AllGather", AL.bypass, ins=[lp1_loc[:]], outs=[lp1_full[:]],
                    replica_groups=[list(range(NCORES))])
            embed("ce_", "consT_aug", CF, NCLp, NCLg, c0T,
                  [("vc_wr", None, "rp1_loc", NCL)])

            # =========== conv (edges + interleaved feature-major post) =====
            def conv(cv, pre, lp_dram, rp_dram, src_d, oh_d, ohT_d, rightT, outT,
                     rcnt_name, ind_name, projs, n_valid, post_cb=None):
                nblocks = cv.nblocks
                ngroups = -(-nblocks // 4)
                grp_ps = [None] * ngroups
                grp_done = [0] * ngroups

                def post_group(g):
                    lo = g * 512
                    sl = slice(lo, lo + 512)
                    psG = grp_ps[g]
                    mean = wpool.tile([128, 512], dt.bfloat16, tag="pmean")
                    if psG is None:
                        nc.vector.memset(mean[:], 0.0)
                    else:
                        rcb = bc_pool.tile([128, 512], dt.float32, tag="bc")
                        nc.tensor.matmul(rcb[:], cw["ones_row"][:],
                                         cw[rcnt_name][:, sl], start=True,
                                         stop=True)
                        acc_sb = wpool.tile([128, 512], dt.bfloat16, tag="paccsb")
                        nc.vector.tensor_copy(acc_sb[:], psG[:])
                        grp_ps[g] = None
                        nc.vector.tensor_tensor(mean[:], acc_sb[:], rcb[:],
                                                AL.mult)
                    psU = mm_pool.tile([128, 512], dt.float32, tag="mm")
                    nc.tensor.matmul(psU[:], cw[pre + "wf"][:], mean[:],
                                     start=True, stop=False)
                    nc.tensor.matmul(psU[:], cw[pre + "bf_row"][:],
                                     cw[ind_name][:, sl], start=False, stop=True)
                    pst = bc_pool.tile([128, 512], dt.float32, tag="bc")
                    nc.tensor.matmul(pst[0:1, :], cw[pre + "wfbar"][:], mean[:],
                                     start=True, stop=False)
                    nc.tensor.matmul(pst[0:1, :], cw[pre + "bfbar1"][:],
                                     cw[ind_name][:, sl], start=False, stop=True)
                    u_sb = wpool.tile([128, 512], dt.bfloat16, tag="pusb")
                    nc.vector.tensor_copy(u_sb[:], psU[:])
                    usq = wpool.tile([128, 512], dt.bfloat16, tag="pmean")
                    nc.vector.tensor_tensor(usq[:], u_sb[:], u_sb[:], AL.mult)
                    pst2 = bc_pool.tile([128, 512], dt.float32, tag="bc")
                    nc.tensor.matmul(pst2[0:1, :], cw["invemb_col"][:], usq[:],
                                     start=True, stop=True)
                    strow = tpool.tile([1, 512], dt.float32, tag="strow")
                    nc.vector.tensor_copy(strow[0:1, :], pst[0:1, :])
                    strow2 = tpool.tile([1, 512], dt.float32, tag="strow2")
                    nc.vector.tensor_copy(strow2[0:1, :], pst2[0:1, :])
                    rstd_bf, nmur, _ = rowmath_rstd(strow[0:1, :], strow2[0:1, :])
                    rb = bc_pool.tile([128, 512], dt.float32, tag="bc")
                    nc.tensor.matmul(rb[:], cw["ones_row"][:], rstd_bf[:],
                                     start=True, stop=True)
                    t1 = wpool.tile([128, 512], dt.bfloat16, tag="pt1")
                    nc.vector.tensor_tensor(t1[:], u_sb[:], rb[:], AL.mult)
                    psB = mm_pool.tile([128, 512], dt.float32, tag="mm")
                    nc.tensor.matmul(psB[:], cw[pre + "wo1a"][:], t1[:],
                                     start=True, stop=False)
                    nc.tensor.matmul(psB[:], cw[pre + "wo1abar"][:], nmur[:],
                                     start=False, stop=False)
                    nc.tensor.matmul(psB[:], cw[pre + "wo1b"][:], rightT[:, sl],
                                     start=False, stop=True)
                    h2 = wpool.tile([128, 512], dt.bfloat16, tag="ph2")
                    nc.scalar.activation(h2[:], psB[:], LR, bias=cw[pre + "bo1"][:],
                                         alpha=SLOPE)
                    psC = mm_pool.tile([128, 512], dt.float32, tag="mm")
                    nc.tensor.matmul(psC[:], cw[pre + "wo2"][:], h2[:],
                                     start=True, stop=True)
                    nc.vector.tensor_scalar(outT[:, sl], psC[:], 1.0,
                                            cw[pre + "bo2"][:], AL.mult, AL.add)
                    for (wname, brow, dout) in projs:
                        for bi in range(4):
                            blo = lo + bi * 128
                            nv = min(128, max(0, n_valid - blo))
                            if nv == 0:
                                continue
                            psP = mm_pool.tile([128, 512], dt.float32, tag="mm")
                            nc.tensor.matmul(psP[:, :128], outT[:, blo : blo + 128],
                                             cw[wname][:], start=True, stop=True)
                            ob = wpool.tile([128, EMB], dt.bfloat16, tag="pob")
                            if brow is not None:
                                nc.vector.tensor_tensor(ob[:], psP[:, :128],
                                                        cw[brow][:], AL.add)
                            else:
                                nc.vector.tensor_copy(ob[:], psP[:, :128])
                            nc.sync.dma_start(b.d[dout][blo : blo + nv, :],
                                              ob[:nv, :])
                    if post_cb is not None:
                        post_cb(g)

                cur_rp = [None, -1]

                def get_rp(blk):
                    if cur_rp[1] == blk:
                        return cur_rp[0]
                    rp_sb = wpool.tile([128, EMB], dt.bfloat16, tag="rpblk")
                    lo = blk * 128
                    nv = min(128, n_valid - lo)
                    if nv < 128:
                        nc.vector.memset(rp_sb[:], 0.0)
                    nc.sync.dma_start(rp_sb[:nv, :], rp_dram[lo : lo + nv, :])
                    cur_rp[0] = rp_sb
                    cur_rp[1] = blk
                    return rp_sb

                for si, seg in enumerate(cv.segments):
                    blk = seg["blk"]
                    g = blk // 4
                    seg_first = (si == 0 or cv.segments[si - 1]["blk"] != blk)
                    seg_last = (si + 1 == len(cv.segments)
                                or cv.segments[si + 1]["blk"] != blk)
                    base_edge = seg["start_edge"]
                    ntiles = seg["ntiles"]
                    view_lo = HI_BASE if seg["u"] == 1 else 0
                    lp_view = lp_dram[view_lo:, :] if view_lo else lp_dram[:, :]
                    rp_sb = get_rp(blk)

                    tdone = 0
                    while tdone < ntiles:
                        tcn = min(CHUNK_TILES, ntiles - tdone)
                        e0 = base_edge + tdone * 128
                        ne = tcn * 128
                        sidx = gpool.tile([128, CHUNK_TILES * 8], dt.int16,
                                          tag="sidx")
                        nc.sync.dma_start(sidx[:, : ne // 16],
                                          src_d[:, e0 // 16 : (e0 + ne) // 16])
                        gbuf = gpool.tile([128, CHUNK_TILES, EMB], dt.bfloat16,
                                          tag="sgat")
                        nc.gpsimd.dma_gather(gbuf[:, :tcn, :], lp_view,
                                             sidx[:, : ne // 16], ne, ne, EMB,
                                             single_packet=False)
                        ohe = gpool.tile([128, CHUNK_TILES * 128], dt.float8e4,
                                         tag="ohe")
                        nc.sync.dma_start(ohe[:, :ne], oh_d[:, e0 : e0 + ne])
                        ohT = gpool.tile([128, CHUNK_TILES * 128], dt.float8e4,
                                         tag="ohT")
                        nc.sync.dma_start(ohT[:, :ne], ohT_d[:, e0 : e0 + ne])

                        xw_c = gpool.tile([128, CHUNK_TILES, EMB], dt.bfloat16,
                                          tag="xwc")
                        st6 = tpool.tile([128, CHUNK_TILES, 6], dt.float32,
                                         tag="st6")
                        mv_c = tpool.tile([128, CHUNK_TILES, 2], dt.float32,
                                          tag="mvc")

                        ngrp4 = -(-tcn // 4)
                        for g4 in range(ngrp4):
                            lo4 = g4 * 4
                            n4 = min(4, tcn - lo4)
                            psx = psx_pool.tile([128, 512], dt.float32, tag="psx")
                            for i in range(n4):
                                ti = lo4 + i
                                nc.tensor.matmul(
                                    psx[:, i * 128 : (i + 1) * 128],
                                    ohT[:, ti * 128 : (ti + 1) * 128], rp_sb[:],
                                    start=True, stop=True)
                            nc.vector.tensor_tensor(
                                xw_c[:, lo4 : lo4 + n4, :],
                                psx[:, : n4 * 128],
                                gbuf[:, lo4 : lo4 + n4, :], AL.add)
                            for i in range(n4):
                                ti = lo4 + i
                                nc.vector.bn_stats(st6[:, ti, :], xw_c[:, ti, :])
                                nc.vector.bn_aggr(mv_c[:, ti, :], st6[:, ti, :])

                        veps = tpool.tile([128, CHUNK_TILES], dt.float32,
                                          tag="vepsc")
                        nc.vector.tensor_scalar(veps[:, :tcn], mv_c[:, :tcn, 1],
                                                EPS, None, AL.add)
                        sdc = tpool.tile([128, CHUNK_TILES], dt.float32, tag="sdc")
                        nc.scalar.activation(sdc[:, :tcn], veps[:, :tcn], SQ)
                        rstd_t = tpool.tile([128, CHUNK_TILES], dt.float32,
                                            tag="rstdc")
                        nc.vector.reciprocal(rstd_t[:, :tcn], sdc[:, :tcn])
                        nmr_c = tpool.tile([128, CHUNK_TILES], dt.float32,
                                           tag="nmrc")
                        nc.vector.scalar_tensor_tensor(
                            nmr_c[:, :tcn], mv_c[:, :tcn, 0], -1.0,
                            rstd_t[:, :tcn], AL.mult, AL.mult)

                        for ti in range(tcn):
                            act = wpool.tile([128, EMB], dt.bfloat16, tag="act")
                            nc.scalar.activation(
                                act[:], xw_c[:, ti, :], LR,
                                bias=nmr_c[:, ti : ti + 1],
                                scale=rstd_t[:, ti : ti + 1], alpha=SLOPE)
                            if grp_ps[g] is None:
                                agg_t = agg_pool.tile([128, 512], dt.float32,
                                                      tag="agg")
                                grp_ps[g] = agg_t
                            first = seg_first and tdone == 0 and ti == 0
                            last = seg_last and (tdone + ti + 1 == ntiles)
                            bslot = blk % 4
                            nc.tensor.matmul(
                                grp_ps[g][:, bslot * 128 : (bslot + 1) * 128],
                                act[:], ohe[:, ti * 128 : (ti + 1) * 128],
                                start=first, stop=last)
                        tdone += tcn

                    if seg_last:
                        grp_done[g] += 1
                        gnb = min(4, nblocks - g * 4)
                        if grp_done[g] == gnb:
                            post_group(g)

                for g in range(ngroups):
                    gnb = min(4, nblocks - g * 4)
                    if grp_done[g] < gnb:
                        post_group(g)

            # =========== heads ===========
            def heads_chunk(j):
                sl = slice(j * 512, (j + 1) * 512)
                if nact == 0:
                    nc.vector.memset(outrow[:, sl], 0.0)
                    return
                pso = bc_pool.tile([128, 512], dt.float32, tag="bc")
                for hi in range(nact):
                    psH = mm_pool.tile([128, 512], dt.float32, tag="mm")
                    nc.tensor.matmul(psH[:], cw["hw1"][:, hi, :], v1T[:, sl],
                                     start=True, stop=True)
                    hh = wpool.tile([128, 512], dt.bfloat16, tag="hh")
                    nc.scalar.activation(hh[:], psH[:], LR,
                                         bias=cw["hb1"][:, hi : hi + 1],
                                         alpha=SLOPE)
                    nc.tensor.matmul(pso[0:1, :], cw["hw2"][:, hi : hi + 1], hh[:],
                                     start=(hi == 0), stop=(hi == nact - 1))
                nc.vector.tensor_scalar(outrow[:, sl], pso[0:1, :],
                                        p["out_scale"], p["out_add"],
                                        AL.mult, AL.add)

            # =========== run ===========
            if KSTAGE == "A":
                nc.vector.memset(outrow[:], 0.0)
            else:
                conv(c1p, "vc_", lp1_full, rp1_loc, b.d["e1_src"],
                     b.d["e1_oh"], b.d["e1_ohT"], c0T, c1T, "rcnt1", "ind1",
                     [("cv_wl", "cv_bl_row", "lp2_loc")], NCL)
                if KSTAGE == "C1":
                    nc.vector.memset(outrow[:], 0.0)
                else:
                    nc.gpsimd.collective_compute(
                        "AllGather", AL.bypass, ins=[lp2_loc[:]],
                        outs=[lp2_full[:]], replica_groups=[list(range(NCORES))])
                    heads_done = set()

                    def post2_cb(g):
                        if g not in heads_done:
                            heads_done.add(g)
                            heads_chunk(g)

                    conv(c2p, "cv_", lp2_full, rp2_loc, b.d["e2_src"],
                         b.d["e2_oh"], b.d["e2_ohT"], v0T, v1T, "rcnt2", "ind2",
                         [], NVL, post_cb=post2_cb)
                    for j in range(NVLg // 512):
                        if j not in heads_done:
                            heads_chunk(j)
            nc.sync.dma_start(out_d[:], outrow[:])

    nc.compile()
    return b


_CACHE = {}


def kernel(**inputs):
    key = tuple(sorted((k, tuple(np.asarray(v).shape)) for k, v in inputs.items()))
    p = host_prep(inputs)
    ck = (key, p["nact"], p["conv1"].etot, p["conv2"].etot)
    if ck in _CACHE:
        b = _CACHE[ck]
    else:
        b = build_program(p)
        _CACHE[ck] = b
    in_maps = [dict(p["core_inputs"][c]) for c in range(NCORES)]
    res = run_bass_kernel_spmd(b.nc, in_maps, core_ids=list(range(NCORES)))
    NVL = p["NVL"]
    out = np.concatenate([res.results[c]["out"][0, :NVL] for c in range(NCORES)])
    return out.astype(np.float32)
